# revision 6
# baseline (speedup 1.0000x reference)
"""Trainium2 Bass kernel for nn_ActorCriticTransformer (sparse_attention).

Strategy: pure data-parallel over batch (16 samples / 8 cores = 2 per core).
Per-core: conv feature extractor + 4 PyraFormer blocks + actor/critic heads,
computed feature-major ([D partitions, tokens free]) in bf16 with f32 PSUM
accumulation and an f32 residual stream.

Key algebraic tricks (validated in numpy golden sim):
 - LayerNorm gamma/beta folded into adjacent projection weights host-side;
   on-chip LN reduces to stats + broadcast + 2 DVE ops.
 - Attention computed in transposed-score layout  sT[t,s] = K @ Q^T  so no
   PE transposes are needed anywhere; V is produced token-major directly by
   swapping matmul operand roles; a ones-column appended to V yields softmax
   row-sums for free; softmax normalization deferred to a column-scaling of
   the per-head output (commutes with Wo).
 - Softmax computed without max-subtraction (inputs are bounded); sparse
   masks applied multiplicatively post-exp ({0,1} masks resident in SBUF).
"""
import os
import numpy as np
import ml_dtypes

import concourse.bass as bass
import concourse.mybir as mybir
import concourse.tile as tile
from concourse.vector_clock import ScopedClock
from concourse.bass_utils import run_bass_kernel_spmd

BF16 = ml_dtypes.bfloat16
f32 = mybir.dt.float32
bf16 = mybir.dt.bfloat16
AF = mybir.ActivationFunctionType
OP = mybir.AluOpType

B, S, F, D, H, L, A = 16, 512, 64, 512, 8, 4, 3
E = D // H
NC = 8
BPC = B // NC
N = BPC * S
P = 128

# ---------------------------------------------------------------------------
# walrus in this toolchain supports only ONE sync wait per instruction.
# Patch the Tile tail drain and add a global post-pass splitting extra waits
# onto fresh same-engine nops.
# ---------------------------------------------------------------------------
_DMA_TYPES = tuple(
    t for t in (
        getattr(mybir, "InstTensorLoad", None),
        getattr(mybir, "InstTensorSave", None),
        getattr(mybir, "InstDmaTrigger", None),
        getattr(mybir, "InstTensorLoadIndirect", None),
        getattr(mybir, "InstTensorSaveIndirect", None),
    ) if t is not None
)
_wsplit_counter = [0]


def split_sync_waits(nc, max_waits=1):
    n_split = 0
    for fn in nc.m.functions:
        for bb in fn.blocks:
            if not any(
                i.sync_info is not None and i.sync_info.on_wait
                and len(i.sync_info.on_wait) > max_waits
                and not isinstance(i, _DMA_TYPES)
                for i in bb.instructions
            ):
                continue
            newlist = []
            for inst in bb.instructions:
                si = inst.sync_info
                if si is not None and si.on_wait and len(si.on_wait) > max_waits \
                        and not isinstance(inst, _DMA_TYPES):
                    waits = list(si.on_wait)
                    extra = waits[max_waits:]
                    del si.on_wait[max_waits:]
                    for i in range(0, len(extra), max_waits):
                        _wsplit_counter[0] += 1
                        n_split += 1
                        newlist.append(mybir.InstNoOp(
                            name=f"I-wsplit-{_wsplit_counter[0]}",
                            engine=inst.engine, ins=[], outs=[],
                            sync_info=mybir.SyncInfo(
                                on_wait=extra[i:i + max_waits], on_update=[]),
                            text_hint="wsplit", bass_nofuse=True))
                newlist.append(inst)
            try:
                bb.instructions[:] = newlist
            except TypeError:
                bb.instructions = newlist
    return n_split


def _patched_drain_and_barrier(self, tick_clock, wait_clock):
    nc = self.nc
    drain_inst = nc.sync.drain()
    wait_clock.add_sem_waits(
        drain_inst.ins, ScopedClock({None: tick_clock.global_clock}))
    nc.all_engine_barrier()
    assert self.sems is not None
    popped = nc._tile_sem_poison_stack.pop()
    assert popped is self._sem_poison
    nc.clear_and_free_semaphores(list(self.sems.allocated().values()))
    nc.all_engine_barrier()


tile.TileContext._drain_and_barrier = _patched_drain_and_barrier


# ---------------------------------------------------------------------------
# Host-side preprocessing
# ---------------------------------------------------------------------------
def _bf(x):
    return np.ascontiguousarray(np.asarray(x, np.float32)).astype(BF16)


def _f32(x):
    return np.ascontiguousarray(np.asarray(x, np.float32))


def make_masks():
    i = np.arange(S)[:, None]
    j = np.arange(S)[None, :]
    d = np.abs(i - j)
    ms = []
    for h in range(5):
        dil = 2 ** h
        ms.append(((d <= dil) | (d % (2 ** (h + 1)) == 0)).astype(np.float32))
    return np.stack(ms)  # [5,S,S]


def preprocess(inp):
    w = {}
    c1 = np.asarray(inp["conv1_w"], np.float32)          # [256,64,3]
    w["c1w"] = _bf(c1.transpose(1, 2, 0).reshape(64, 3, 2, 128))
    c2 = np.asarray(inp["conv2_w"], np.float32)          # [512,256,3]
    # [p,dk,tap,mo,m] = conv2_w[mo*128+m, dk*128+p, tap]
    c2w = np.zeros((128, 2, 3, 4, 128), np.float32)
    for dk in range(2):
        for tap in range(3):
            for mo in range(4):
                c2w[:, dk, tap, mo, :] = c2[mo * 128:(mo + 1) * 128,
                                            dk * 128:(dk + 1) * 128, tap].T
    w["c2w"] = _bf(c2w)
    embw = np.asarray(inp["emb_w"], np.float32)          # [512,512]
    e4 = np.zeros((128, 4, 4, 128), np.float32)
    for dk in range(4):
        for mo in range(4):
            e4[:, dk, mo, :] = embw[mo * 128:(mo + 1) * 128,
                                    dk * 128:(dk + 1) * 128].T
    w["embw"] = _bf(e4)
    pos = np.asarray(inp["pos"], np.float32)[0]          # [S,D]
    w["post"] = _bf(pos.T.reshape(4, 128, 512).transpose(1, 0, 2))
    mk = make_masks()                                    # [5,S,S]
    w["masks"] = _bf(mk.reshape(5, 4, 128, 512).transpose(2, 0, 1, 3))
    # conv/emb biases: [128, 10] cols: c1b(2) c2b(4) embb(4)
    b0 = np.zeros((128, 10), np.float32)
    b0[:, 0:2] = _f32(inp["conv1_b"]).reshape(2, 128).T
    b0[:, 2:6] = _f32(inp["conv2_b"]).reshape(4, 128).T
    b0[:, 6:10] = _f32(inp["emb_b"]).reshape(4, 128).T
    w["bias0"] = b0

    wq_a = np.zeros((L, 128, 4, 4, 128), np.float32)
    wk_a = np.zeros((L, 128, 4, 4, 128), np.float32)
    wv_a = np.zeros((L, 128, 4, 512), np.float32)
    wo_a = np.zeros((L, 128, 4, 512), np.float32)
    w1_a = np.zeros((L, 128, 4, 2048), np.float32)
    w2_a = np.zeros((L, 128, 16, 512), np.float32)
    lb_a = np.zeros((L, 128, 32), np.float32)
    for l in range(L):
        g1 = np.asarray(inp["ln1_g"][l], np.float32)
        b1v = np.asarray(inp["ln1_b"][l], np.float32)
        g2 = np.asarray(inp["ln2_g"][l], np.float32)
        b2v = np.asarray(inp["ln2_b"][l], np.float32)
        Wq = np.asarray(inp["Wq"][l], np.float32)        # [H,E,D]
        Wk = np.asarray(inp["Wk"][l], np.float32)
        Wv = np.asarray(inp["Wv"][l], np.float32)
        Wo = np.asarray(inp["Wo"][l], np.float32)        # [D,D]
        bq = _f32(inp["bq"][l]) + np.einsum("hed,d->he", Wq, b1v)
        bk = _f32(inp["bk"][l]) + np.einsum("hed,d->he", Wk, b1v)
        bv = _f32(inp["bv"][l]) + np.einsum("hed,d->he", Wv, b1v)
        Wqg = Wq * g1[None, None, :]
        Wkg = Wk * g1[None, None, :]
        Wvg = Wv * g1[None, None, :]
        for pr in range(4):
            qT = np.concatenate([Wqg[2 * pr].T, Wqg[2 * pr + 1].T], 1)  # [D,128]
            kT = np.concatenate([Wkg[2 * pr].T, Wkg[2 * pr + 1].T], 1)
            for dk in range(4):
                wq_a[l, :, pr, dk, :] = qT[dk * 128:(dk + 1) * 128]
                wk_a[l, :, pr, dk, :] = kT[dk * 128:(dk + 1) * 128]
            lb_a[l, :, 0 + pr] = np.concatenate([bq[2 * pr], bq[2 * pr + 1]])
            lb_a[l, :, 4 + pr] = np.concatenate([bk[2 * pr], bk[2 * pr + 1]])
        vT = Wvg.transpose(2, 0, 1).reshape(D, H * E)    # [D, 512]
        woT = Wo.T                                       # [D_in, D_out]
        bo = _f32(inp["bo"][l]) + Wo @ bv.reshape(-1)
        W1g = np.asarray(inp["W1"][l], np.float32) * g2[None, :]
        b1f = _f32(inp["b1"][l]) + np.asarray(inp["W1"][l], np.float32) @ b2v
        W2 = np.asarray(inp["W2"][l], np.float32)
        for dk in range(4):
            wv_a[l, :, dk, :] = vT[dk * 128:(dk + 1) * 128]
            wo_a[l, :, dk, :] = woT[dk * 128:(dk + 1) * 128]
            w1_a[l, :, dk, :] = W1g.T[dk * 128:(dk + 1) * 128]
        for hk in range(16):
            w2_a[l, :, hk, :] = W2.T[hk * 128:(hk + 1) * 128]
        lb_a[l, :, 8:12] = bo.reshape(4, 128).T
        lb_a[l, :, 12:28] = b1f.reshape(16, 128).T
        lb_a[l, :, 28:32] = _f32(inp["b2"][l]).reshape(4, 128).T
    w["wq"], w["wk"], w["wv"], w["wo"] = map(_bf, (wq_a, wk_a, wv_a, wo_a))
    w["w1"], w["w2"] = _bf(w1_a), _bf(w2_a)
    w["lbias"] = lb_a

    # heads (lnf folded into layer 1; a3/c3 padded 57 -> 64)
    gf = np.asarray(inp["lnf_g"], np.float32)
    bfv = np.asarray(inp["lnf_b"], np.float32)
    hw1 = np.zeros((128, 2, 4, 128), np.float32)
    hw2 = np.zeros((128, 2, 2, 128), np.float32)
    hw3 = np.zeros((128, 2, 2, 64), np.float32)
    hw4 = np.zeros((64, 2, 3), np.float32)
    hb1 = np.zeros((128, 2), np.float32)
    hb2 = np.zeros((128, 2, 2), np.float32)
    hb3 = np.zeros((64, 2), np.float32)
    hb4 = np.zeros((3, 1), np.float32)
    hb4c = np.zeros((1, 1), np.float32)
    for ai, nm in enumerate(("a", "c")):
        w1h = np.asarray(inp[f"{nm}1_w"], np.float32) * gf[None, :]   # [128,512]
        b1h = _f32(inp[f"{nm}1_b"]) + np.asarray(inp[f"{nm}1_w"], np.float32) @ bfv
        for dk in range(4):
            hw1[:, ai, dk, :] = w1h.T[dk * 128:(dk + 1) * 128]
        hb1[:, ai] = b1h
        w2h = np.asarray(inp[f"{nm}2_w"], np.float32)                 # [256,128]
        for mo in range(2):
            hw2[:, ai, mo, :] = w2h[mo * 128:(mo + 1) * 128].T
        hb2[:, ai, :] = _f32(inp[f"{nm}2_b"]).reshape(2, 128).T
        w3h = np.asarray(inp[f"{nm}3_w"], np.float32)                 # [57,256]
        for dk in range(2):
            hw3[:, ai, dk, 0:57] = w3h[:, dk * 128:(dk + 1) * 128].T
        hb3[0:57, ai] = _f32(inp[f"{nm}3_b"])
        w4h = np.asarray(inp[f"{nm}4_w"], np.float32)                 # [A or 1, 57]
        if nm == "a":
            hw4[0:57, 0, :] = w4h.T
            hb4[:, 0] = _f32(inp["a4_b"])
        else:
            hw4[0:57, 1, 0:1] = w4h.T
            hb4c[:, 0] = _f32(inp["c4_b"])
    w["hw1"], w["hw2"], w["hw3"], w["hw4"] = map(_bf, (hw1, hw2, hw3, hw4))
    w["hb1"], w["hb2"], w["hb3"], w["hb4"], w["hb4c"] = hb1, hb2, hb3, hb4, hb4c
    return w


def prep_x(x):
    """x [B,S,F] -> per-core padded feature-major bf16 [64, BPC, 514]."""
    outs = []
    for c in range(NC):
        xp = np.zeros((F, BPC, S + 2), np.float32)
        xp[:, :, 1:S + 1] = np.asarray(
            x[c * BPC:(c + 1) * BPC], np.float32).transpose(2, 0, 1)
        outs.append(_bf(xp))
    return outs


# ---------------------------------------------------------------------------
# Bass kernel builder
# ---------------------------------------------------------------------------
def build_kernel(dbg_stage=""):
    nc = bass.Bass()

    def par(name, shape, dt=bf16):
        return nc.declare_dram_parameter(name, list(shape), dt, isOutput=False)

    xpad_d = par("xpad", [64, BPC, 514])
    c1w_d = par("c1w", [64, 3, 2, 128])
    c2w_d = par("c2w", [128, 2, 3, 4, 128])
    embw_d = par("embw", [128, 4, 4, 128])
    post_d = par("post", [128, 4, 512])
    masks_d = par("masks", [128, 5, 4, 512])
    bias0_d = par("bias0", [128, 10], f32)
    wq_d = par("wq", [L, 128, 4, 4, 128])
    wk_d = par("wk", [L, 128, 4, 4, 128])
    wv_d = par("wv", [L, 128, 4, 512])
    wo_d = par("wo", [L, 128, 4, 512])
    w1_d = par("w1", [L, 128, 4, 2048])
    w2_d = par("w2", [L, 128, 16, 512])
    lbias_d = par("lbias", [L, 128, 32], f32)
    hw1_d = par("hw1", [128, 2, 4, 128])
    hw2_d = par("hw2", [128, 2, 2, 128])
    hw3_d = par("hw3", [128, 2, 2, 64])
    hw4_d = par("hw4", [64, 2, 3])
    hb1_d = par("hb1", [128, 2], f32)
    hb2_d = par("hb2", [128, 2, 2], f32)
    hb3_d = par("hb3", [64, 2], f32)
    hb4_d = par("hb4", [3, 1], f32)
    hb4c_d = par("hb4c", [1, 1], f32)
    out_d = nc.declare_dram_parameter("out", [4, BPC], f32, isOutput=True)
    dbg_d = None
    if dbg_stage:
        dbg_d = nc.declare_dram_parameter("dbg", [128, 4, N], f32, isOutput=True)

    with tile.TileContext(nc) as tc:
        with tc.tile_pool(name="single", bufs=1) as single, \
             tc.tile_pool(name="wpool", bufs=2) as wpool, \
             tc.tile_pool(name="big1", bufs=1) as big1, \
             tc.tile_pool(name="per2", bufs=2) as per2, \
             tc.tile_pool(name="per3", bufs=3) as per3, \
             tc.tile_pool(name="ps", bufs=1, space="PSUM") as ps:

            # ---------------- persistent tiles ----------------
            x_sb = single.tile([64, BPC, 514], bf16)
            nc.sync.dma_start(out=x_sb, in_=xpad_d[:])
            c1w_t = single.tile([64, 3, 2, 128], bf16)
            nc.sync.dma_start(out=c1w_t, in_=c1w_d[:])
            c2w_t = single.tile([128, 2, 3, 4, 128], bf16)
            nc.sync.dma_start(out=c2w_t, in_=c2w_d[:])
            embw_t = single.tile([128, 4, 4, 128], bf16)
            nc.sync.dma_start(out=embw_t, in_=embw_d[:])
            pos_t = single.tile([128, 4, 512], bf16)
            nc.sync.dma_start(out=pos_t, in_=post_d[:])
            mask_t = single.tile([128, 5, 4, 512], bf16)
            nc.sync.dma_start(out=mask_t, in_=masks_d[:])
            bias0_t = single.tile([128, 10], f32)
            nc.sync.dma_start(out=bias0_t, in_=bias0_d[:])

            ones128 = single.tile([128, 1], bf16)   # stats lhsT
            nc.vector.memset(ones128, 1.0)
            ones1 = single.tile([1, 128], bf16)     # bcast lhsT
            nc.vector.memset(ones1, 1.0)
            nones = single.tile([1, 128], bf16)     # bcast lhsT * (-1/512)
            nc.vector.memset(nones, -1.0 / 512.0)
            eps1 = single.tile([1, 1], f32)
            nc.vector.memset(eps1, 1e-5)

            zt = big1.tile([128, 4, N], f32)        # residual stream
            h1_sb = big1.tile([128, 2, BPC, 514], bf16)
            nc.vector.memset(h1_sb, 0.0)
            h2_sb = big1.tile([128, 4, BPC, 512], bf16)

            # ---------------- conv feature extractor ----------------
            for s in range(BPC):
                for mc in range(2):
                    pc = ps.tile([128, 512], f32, tag="mm", bufs=2)
                    for d in range(3):
                        nc.tensor.matmul(pc, c1w_t[:, d, mc, :],
                                         x_sb[:, s, d:d + 512],
                                         start=(d == 0), stop=(d == 2))
                    nc.scalar.activation(out=h1_sb[:, mc, s, 1:513], in_=pc,
                                         func=AF.Gelu, bias=bias0_t[:, mc:mc + 1])
            for s in range(BPC):
                for mo in range(4):
                    pc = ps.tile([128, 512], f32, tag="mm", bufs=2)
                    k = 0
                    for dk in range(2):
                        for d in range(3):
                            nc.tensor.matmul(pc, c2w_t[:, dk, d, mo, :],
                                             h1_sb[:, dk, s, d:d + 512],
                                             start=(k == 0), stop=(k == 5))
                            k += 1
                    nc.scalar.activation(out=h2_sb[:, mo, s, :], in_=pc,
                                         func=AF.Gelu,
                                         bias=bias0_t[:, 2 + mo:3 + mo])
            # ---------------- input embedding ----------------
            for s in range(BPC):
                for mo in range(4):
                    pc = ps.tile([128, 512], f32, tag="mm", bufs=2)
                    for dk in range(4):
                        nc.tensor.matmul(pc, embw_t[:, dk, mo, :],
                                         h2_sb[:, dk, s, :],
                                         start=(dk == 0), stop=(dk == 3))
                    nc.vector.scalar_tensor_tensor(
                        out=zt[:, mo, s * S:(s + 1) * S], in0=pc,
                        scalar=bias0_t[:, 6 + mo:7 + mo], in1=pos_t[:, mo, :],
                        op0=OP.add, op1=OP.add)

            # ---------------- LN helper ----------------
            def layer_norm(dst_u, s):
                """LN over feature dim of zt columns of sample s -> dst_u
                (bf16 [128,4,N]); stats via ones-matmuls; rstd via ln/exp."""
                sl = slice(s * S, (s + 1) * S)
                zbf = per2.tile([128, 4, S], bf16, tag="zbf", bufs=1)
                zsq = per2.tile([128, 4, S], bf16, tag="zsq", bufs=1)
                for c in range(4):
                    nc.vector.tensor_copy(out=zbf[:, c, :], in_=zt[:, c, sl])
                    nc.scalar.activation(out=zsq[:, c, :], in_=zt[:, c, sl],
                                         func=AF.Square)
                p_sz = ps.tile([1, 512], f32, tag="st", bufs=2)
                p_sq = ps.tile([1, 512], f32, tag="st", bufs=2)
                for c in range(4):
                    nc.tensor.matmul(p_sz, ones128, zbf[:, c, :],
                                     start=(c == 0), stop=(c == 3))
                for c in range(4):
                    nc.tensor.matmul(p_sq, ones128, zsq[:, c, :],
                                     start=(c == 0), stop=(c == 3))
                sumz = per3.tile([1, 512], f32, tag="sumz", bufs=2)
                nc.vector.tensor_copy(out=sumz, in_=p_sz)
                t2 = per3.tile([1, 512], f32, tag="tchain", bufs=4, name="t2")
                nc.vector.tensor_scalar(out=t2, in0=p_sq, scalar1=512.0,
                                        scalar2=None, op0=OP.mult)
                t1 = per3.tile([1, 512], f32, tag="tchain", bufs=4, name="t1")
                nc.vector.tensor_tensor(out=t1, in0=sumz, in1=sumz, op=OP.mult)
                vr = per3.tile([1, 512], f32, tag="tchain", bufs=4, name="vr")
                nc.vector.tensor_tensor(out=vr, in0=t2, in1=t1, op=OP.subtract)
                lnv = per3.tile([1, 512], f32, tag="tchain", bufs=4, name="lnv")
                nc.scalar.activation(out=lnv, in_=vr, func=AF.Ln,
                                     scale=1.0 / (512.0 * 512.0), bias=eps1)
                s1row = per3.tile([1, 512], bf16, tag="srow", bufs=4, name="s1row")
                nc.scalar.activation(out=s1row, in_=lnv, func=AF.Exp, scale=-0.5)
                s2tmp = per3.tile([1, 512], bf16, tag="srow", bufs=4, name="s2tmp")
                nc.vector.tensor_tensor(out=s2tmp, in0=sumz, in1=s1row,
                                        op=OP.mult)
                p_s1b = ps.tile([128, 512], f32, tag="mm", bufs=2)
                nc.tensor.matmul(p_s1b, ones1, s1row, start=True, stop=True)
                p_s2b = ps.tile([128, 512], f32, tag="mm", bufs=2)
                nc.tensor.matmul(p_s2b, nones, s2tmp, start=True, stop=True)
                for c in range(4):
                    nc.vector.tensor_tensor(out=dst_u[:, c, sl],
                                            in0=zt[:, c, sl],
                                            in1=p_s1b, op=OP.mult)
                    nc.vector.tensor_tensor(out=dst_u[:, c, sl],
                                            in0=dst_u[:, c, sl],
                                            in1=p_s2b, op=OP.add)

            # ---------------- transformer layers ----------------
            for l in range(L):
                wq_t = wpool.tile([128, 4, 4, 128], bf16, tag="wq", bufs=1)
                nc.sync.dma_start(out=wq_t, in_=wq_d[l])
                wk_t = wpool.tile([128, 4, 4, 128], bf16, tag="wk", bufs=1)
                nc.sync.dma_start(out=wk_t, in_=wk_d[l])
                wv_t = wpool.tile([128, 4, 512], bf16, tag="wv", bufs=1)
                nc.sync.dma_start(out=wv_t, in_=wv_d[l])
                wo_t = wpool.tile([128, 4, 512], bf16, tag="wo", bufs=1)
                nc.sync.dma_start(out=wo_t, in_=wo_d[l])
                w1_t = wpool.tile([128, 4, 2048], bf16, tag="w1", bufs=1)
                nc.sync.dma_start(out=w1_t, in_=w1_d[l])
                w2_t = wpool.tile([128, 16, 512], bf16, tag="w2", bufs=1)
                nc.sync.dma_start(out=w2_t, in_=w2_d[l])
                lb_t = wpool.tile([128, 32], f32, tag="lb", bufs=1)
                nc.sync.dma_start(out=lb_t, in_=lbias_d[l])

                u = per2.tile([128, 4, N], bf16, tag="u", bufs=1)
                for s in range(BPC):
                    layer_norm(u, s)

                # QKV projections
                q_all = per2.tile([128, 4, N], bf16, tag="q", bufs=1)
                k_all = per2.tile([128, 4, N], bf16, tag="k", bufs=1)
                for dst, wt, bcol in ((q_all, wq_t, 0), (k_all, wk_t, 4)):
                    for pr in range(4):
                        for half in range(2):
                            pc = ps.tile([128, 512], f32, tag="mm", bufs=2)
                            for dk in range(4):
                                nc.tensor.matmul(
                                    pc, wt[:, pr, dk, :],
                                    u[:, dk, half * 512:(half + 1) * 512],
                                    start=(dk == 0), stop=(dk == 3))
                            nc.scalar.activation(
                                out=dst[:, pr, half * 512:(half + 1) * 512],
                                in_=pc, func=AF.Identity,
                                bias=lb_t[:, bcol + pr:bcol + pr + 1])
                v_aug = per2.tile([128, 8, 8, 65], bf16, tag="vaug", bufs=1)
                nc.vector.memset(v_aug[:, :, :, 64:65], 1.0)
                for tcg in range(8):
                    pc = ps.tile([128, 512], f32, tag="mm", bufs=2)
                    for dk in range(4):
                        nc.tensor.matmul(
                            pc, u[:, dk, tcg * 128:(tcg + 1) * 128],
                            wv_t[:, dk, :], start=(dk == 0), stop=(dk == 3))
                    for h in range(H):
                        nc.vector.tensor_copy(
                            out=v_aug[:, h, tcg, 0:64],
                            in_=pc[:, h * 64:(h + 1) * 64])

                # attention
                o_all = per2.tile([128, 4, N], bf16, tag="oall", bufs=1)
                for s in range(BPC):
                    for pr in range(4):
                        rts = [per3.tile([1, 512], bf16, tag="rt0", name="rt0", bufs=2),
                               per3.tile([1, 512], bf16, tag="rt1", name="rt1", bufs=2)]
                        pvs = []
                        for o in range(2):
                            h = 2 * pr + o
                            ob = o * 64
                            pv = ps.tile([65, 512], f32, tag="pv", bufs=4)
                            pvs.append(pv)
                            for tcl in range(4):
                                pss = ps.tile([128, 512], f32, tag="mm", bufs=2)
                                tg = s * 512 + tcl * 128
                                nc.tensor.matmul(
                                    pss,
                                    k_all[ob:ob + 64, pr, tg:tg + 128],
                                    q_all[ob:ob + 64, pr, s * 512:(s + 1) * 512],
                                    start=True, stop=True)
                                pt = per3.tile([128, 512], bf16, tag="pt")
                                nc.scalar.activation(out=pt, in_=pss,
                                                     func=AF.Exp, scale=0.125)
                                nc.vector.tensor_tensor(
                                    out=pt, in0=pt,
                                    in1=mask_t[:, min(h, 4), tcl, :],
                                    op=OP.mult)
                                nc.tensor.matmul(pv, v_aug[:, h, s * 4 + tcl, :],
                                                 pt, start=(tcl == 0),
                                                 stop=(tcl == 3))
                            with nc.allow_low_precision(
                                    reason="softmax renorm reciprocal in bf16"):
                                nc.vector.reciprocal(
                                    out=rts[o], in_=pv[64:65, :])
                        p_b = ps.tile([128, 512], f32, tag="mm", bufs=2)
                        nc.tensor.matmul(p_b[0:64, :], ones1[:, 0:64],
                                         rts[0], start=True, stop=True)
                        nc.tensor.matmul(p_b[64:128, :], ones1[:, 0:64],
                                         rts[1], start=True, stop=True)
                        b_sb = per3.tile([128, 512], bf16, tag="bsb")
                        nc.scalar.activation(out=b_sb, in_=p_b, func=AF.Identity)
                        for o in range(2):
                            nc.vector.tensor_tensor(
                                out=o_all[o * 64:(o + 1) * 64, pr,
                                          s * 512:(s + 1) * 512],
                                in0=pvs[o][0:64, :],
                                in1=b_sb[o * 64:(o + 1) * 64, :], op=OP.mult)

                # output projection + residual
                for s in range(BPC):
                    for mo in range(4):
                        pc = ps.tile([128, 512], f32, tag="mm", bufs=2)
                        for dk in range(4):
                            nc.tensor.matmul(
                                pc, wo_t[:, dk, mo * 128:(mo + 1) * 128],
                                o_all[:, dk, s * 512:(s + 1) * 512],
                                start=(dk == 0), stop=(dk == 3))
                        nc.vector.scalar_tensor_tensor(
                            out=zt[:, mo, s * S:(s + 1) * S], in0=pc,
                            scalar=lb_t[:, 8 + mo:9 + mo],
                            in1=zt[:, mo, s * S:(s + 1) * S],
                            op0=OP.add, op1=OP.add)

                # FFN
                u2 = per2.tile([128, 4, N], bf16, tag="u", bufs=1)
                for s in range(BPC):
                    layer_norm(u2, s)
                for s in range(BPC):
                    pas = [ps.tile([128, 512], f32, tag="pv", bufs=4,
                                   name=f"pa{mo}") for mo in range(4)]
                    for hc in range(16):
                        pc = ps.tile([128, 512], f32, tag="mm", bufs=2)
                        for dk in range(4):
                            nc.tensor.matmul(
                                pc, w1_t[:, dk, hc * 128:(hc + 1) * 128],
                                u2[:, dk, s * 512:(s + 1) * 512],
                                start=(dk == 0), stop=(dk == 3))
                        hgel = per3.tile([128, 512], bf16, tag="hgel")
                        nc.scalar.activation(out=hgel, in_=pc,
                                             func=AF.Gelu,
                                             bias=lb_t[:, 12 + hc:13 + hc])
                        for mo in range(4):
                            nc.tensor.matmul(
                                pas[mo], w2_t[:, hc, mo * 128:(mo + 1) * 128],
                                hgel, start=(hc == 0), stop=(hc == 15))
                    for mo in range(4):
                        nc.vector.scalar_tensor_tensor(
                            out=zt[:, mo, s * S:(s + 1) * S], in0=pas[mo],
                            scalar=lb_t[:, 28 + mo:29 + mo],
                            in1=zt[:, mo, s * S:(s + 1) * S],
                            op0=OP.add, op1=OP.add)
                if dbg_stage == f"l{l}":
                    dg = per2.tile([128, 4, N], f32, tag="dbg")
                    for c in range(4):
                        nc.vector.tensor_copy(out=dg[:, c, :], in_=zt[:, c, :])
                    nc.sync.dma_start(out=dbg_d[:], in_=dg)
            if dbg_stage == "emb":
                dg = per2.tile([128, 4, N], f32, tag="dbg")
                for c in range(4):
                    nc.vector.tensor_copy(out=dg[:, c, :], in_=zt[:, c, :])
                nc.sync.dma_start(out=dbg_d[:], in_=dg)

            # ---------------- final LN on last-token columns ----------------
            hw1_t = single.tile([128, 2, 4, 128], bf16)
            nc.sync.dma_start(out=hw1_t, in_=hw1_d[:])
            hw2_t = single.tile([128, 2, 2, 128], bf16)
            nc.sync.dma_start(out=hw2_t, in_=hw2_d[:])
            hw3_t = single.tile([128, 2, 2, 64], bf16)
            nc.sync.dma_start(out=hw3_t, in_=hw3_d[:])
            hw4_t = single.tile([64, 2, 3], bf16)
            nc.sync.dma_start(out=hw4_t, in_=hw4_d[:])
            hb1_t = single.tile([128, 2], f32)
            nc.sync.dma_start(out=hb1_t, in_=hb1_d[:])
            hb2_t = single.tile([128, 2, 2], f32)
            nc.sync.dma_start(out=hb2_t, in_=hb2_d[:])
            hb3_t = single.tile([64, 2], f32)
            nc.sync.dma_start(out=hb3_t, in_=hb3_d[:])
            hb4_t = single.tile([3, 1], f32)
            nc.sync.dma_start(out=hb4_t, in_=hb4_d[:])
            hb4c_t = single.tile([1, 1], f32)
            nc.sync.dma_start(out=hb4c_t, in_=hb4c_d[:])

            zf_b = single.tile([128, 4, BPC], bf16)
            zf_q = single.tile([128, 4, BPC], bf16)
            for c in range(4):
                nc.vector.tensor_copy(out=zf_b[:, c, :],
                                      in_=zt[:, c, S - 1:N:S])
                nc.scalar.activation(out=zf_q[:, c, :], in_=zt[:, c, S - 1:N:S],
                                     func=AF.Square)
            pf_sz = ps.tile([1, BPC], f32, tag="st", bufs=2)
            pf_sq = ps.tile([1, BPC], f32, tag="st", bufs=2)
            for c in range(4):
                nc.tensor.matmul(pf_sz, ones128, zf_b[:, c, :],
                                 start=(c == 0), stop=(c == 3))
            for c in range(4):
                nc.tensor.matmul(pf_sq, ones128, zf_q[:, c, :],
                                 start=(c == 0), stop=(c == 3))
            sumzf = single.tile([1, BPC], f32)
            nc.vector.tensor_copy(out=sumzf, in_=pf_sz)
            t2f = single.tile([1, BPC], f32)
            nc.vector.tensor_scalar(out=t2f, in0=pf_sq, scalar1=512.0,
                                    scalar2=None, op0=OP.mult)
            t1f = single.tile([1, BPC], f32)
            nc.vector.tensor_tensor(out=t1f, in0=sumzf, in1=sumzf, op=OP.mult)
            vrf = single.tile([1, BPC], f32)
            nc.vector.tensor_tensor(out=vrf, in0=t2f, in1=t1f, op=OP.subtract)
            lnvf = single.tile([1, BPC], f32)
            nc.scalar.activation(out=lnvf, in_=vrf, func=AF.Ln,
                                 scale=1.0 / (512.0 * 512.0), bias=eps1)
            s1f = single.tile([1, BPC], bf16)
            nc.scalar.activation(out=s1f, in_=lnvf, func=AF.Exp, scale=-0.5)
            s2f = single.tile([1, BPC], bf16)
            nc.vector.tensor_tensor(out=s2f, in0=sumzf, in1=s1f, op=OP.mult)
            pf_s1 = ps.tile([128, BPC], f32, tag="st", bufs=2)
            nc.tensor.matmul(pf_s1, ones1, s1f, start=True, stop=True)
            pf_s2 = ps.tile([128, BPC], f32, tag="st", bufs=2)
            nc.tensor.matmul(pf_s2, nones, s2f, start=True, stop=True)
            feat = single.tile([128, 4, BPC], bf16)
            for c in range(4):
                tmpf = single.tile([128, BPC], f32, tag=f"tmpf{c}")
                nc.vector.tensor_tensor(out=tmpf, in0=zt[:, c, S - 1:N:S],
                                        in1=pf_s1, op=OP.mult)
                nc.vector.tensor_tensor(out=feat[:, c, :], in0=tmpf,
                                        in1=pf_s2, op=OP.add)

            # ---------------- actor/critic heads ----------------
            outs = []
            for ai in range(2):
                pc = ps.tile([128, BPC], f32, tag="st", bufs=2)
                for dk in range(4):
                    nc.tensor.matmul(pc, hw1_t[:, ai, dk, :], feat[:, dk, :],
                                     start=(dk == 0), stop=(dk == 3))
                f1 = single.tile([128, BPC], bf16, tag=f"f1_{ai}")
                nc.scalar.activation(out=f1, in_=pc, func=AF.Gelu,
                                     bias=hb1_t[:, ai:ai + 1])
                f2 = single.tile([128, 2, BPC], bf16, tag=f"f2_{ai}")
                for mo in range(2):
                    pc2 = ps.tile([128, BPC], f32, tag="st", bufs=2)
                    nc.tensor.matmul(pc2, hw2_t[:, ai, mo, :], f1,
                                     start=True, stop=True)
                    nc.scalar.activation(out=f2[:, mo, :], in_=pc2, func=AF.Gelu,
                                         bias=hb2_t[:, ai, mo:mo + 1])
                pc3 = ps.tile([64, BPC], f32, tag="st", bufs=2)
                for dk in range(2):
                    nc.tensor.matmul(pc3, hw3_t[:, ai, dk, :], f2[:, dk, :],
                                     start=(dk == 0), stop=(dk == 1))
                f3 = single.tile([64, BPC], bf16, tag=f"f3_{ai}")
                nc.scalar.activation(out=f3, in_=pc3, func=AF.Gelu,
                                     bias=hb3_t[:, ai:ai + 1])
                outs.append(f3)
            pol_ps = ps.tile([3, BPC], f32, tag="st", bufs=2)
            nc.tensor.matmul(pol_ps, hw4_t[:, 0, :], outs[0],
                             start=True, stop=True)
            pol_sb = single.tile([3, BPC], f32)
            nc.scalar.activation(out=pol_sb, in_=pol_ps, func=AF.Identity,
                                 bias=hb4_t[:, 0:1])
            val_ps = ps.tile([1, BPC], f32, tag="st", bufs=2)
            nc.tensor.matmul(val_ps, hw4_t[:, 1, 0:1], outs[1],
                             start=True, stop=True)
            val_sb = single.tile([1, BPC], f32)
            nc.scalar.activation(out=val_sb, in_=val_ps, func=AF.Identity,
                                 bias=hb4c_t[:, 0:1])
            nc.sync.dma_start(out=out_d[0:3, :], in_=pol_sb)
            nc.sync.dma_start(out=out_d[3:4, :], in_=val_sb)

    n = split_sync_waits(nc)
    return nc


# ---------------------------------------------------------------------------
# Entry point
# ---------------------------------------------------------------------------
_CACHE = {}


def kernel(**inputs):
    dbg_stage = os.environ.get("BASS_DBG_STAGE", "")
    key = ("nc", dbg_stage)
    if key not in _CACHE:
        _CACHE[key] = build_kernel(dbg_stage)
    nc = _CACHE[key]
    w = preprocess(inputs)
    xs = prep_x(np.asarray(inputs["x"], np.float32))
    in_maps = []
    for c in range(NC):
        m = {"xpad": xs[c]}
        m.update(w)
        in_maps.append(m)
    trace = os.environ.get("BASS_KERNEL_TRACE", "") == "1"
    res = run_bass_kernel_spmd(nc, in_maps, core_ids=list(range(NC)),
                               trace=trace)
    kernel.last_result = res
    policy = np.zeros((B, A), np.float32)
    value = np.zeros((B, 1), np.float32)
    for c in range(NC):
        o = np.asarray(res.results[c]["out"], np.float32)
        policy[c * BPC:(c + 1) * BPC] = o[0:3].T
        value[c * BPC:(c + 1) * BPC] = o[3:4].T
    if dbg_stage:
        kernel.dbg = [np.asarray(res.results[c]["dbg"]) for c in range(NC)]
    return policy, value


# revision 8
# speedup vs baseline: 1.0903x; 1.0903x over previous
"""Trainium2 Bass kernel for nn_ActorCriticTransformer (sparse_attention).

Strategy: pure data-parallel over batch (16 samples / 8 cores = 2 per core).
Per-core: conv feature extractor + 4 PyraFormer blocks + actor/critic heads,
computed feature-major ([D partitions, tokens free]) in bf16 with f32 PSUM
accumulation and an f32 residual stream.

Key algebraic tricks (validated in numpy golden sim):
 - LayerNorm gamma/beta folded into adjacent projection weights host-side;
   on-chip LN reduces to stats + broadcast + 2 DVE ops.
 - Attention computed in transposed-score layout  sT[t,s] = K @ Q^T  so no
   PE transposes are needed anywhere; V is produced token-major directly by
   swapping matmul operand roles; a ones-column appended to V yields softmax
   row-sums for free; softmax normalization deferred to a column-scaling of
   the per-head output (commutes with Wo).
 - Softmax computed without max-subtraction (inputs are bounded); sparse
   masks applied multiplicatively post-exp ({0,1} masks resident in SBUF).
"""
import os
import numpy as np
import ml_dtypes

import concourse.bass as bass
import concourse.mybir as mybir
import concourse.tile as tile
from concourse.vector_clock import ScopedClock
from concourse.bass_utils import run_bass_kernel_spmd

BF16 = ml_dtypes.bfloat16
f32 = mybir.dt.float32
bf16 = mybir.dt.bfloat16
AF = mybir.ActivationFunctionType
OP = mybir.AluOpType

B, S, F, D, H, L, A = 16, 512, 64, 512, 8, 4, 3
E = D // H
NC = 8
BPC = B // NC
N = BPC * S
P = 128

# ---------------------------------------------------------------------------
# walrus in this toolchain supports only ONE sync wait per instruction.
# Patch the Tile tail drain and add a global post-pass splitting extra waits
# onto fresh same-engine nops.
# ---------------------------------------------------------------------------
_DMA_TYPES = tuple(
    t for t in (
        getattr(mybir, "InstTensorLoad", None),
        getattr(mybir, "InstTensorSave", None),
        getattr(mybir, "InstDmaTrigger", None),
        getattr(mybir, "InstTensorLoadIndirect", None),
        getattr(mybir, "InstTensorSaveIndirect", None),
    ) if t is not None
)
_wsplit_counter = [0]


def split_sync_waits(nc, max_waits=1):
    n_split = 0
    for fn in nc.m.functions:
        for bb in fn.blocks:
            if not any(
                i.sync_info is not None and i.sync_info.on_wait
                and len(i.sync_info.on_wait) > max_waits
                and not isinstance(i, _DMA_TYPES)
                for i in bb.instructions
            ):
                continue
            newlist = []
            for inst in bb.instructions:
                si = inst.sync_info
                if si is not None and si.on_wait and len(si.on_wait) > max_waits \
                        and not isinstance(inst, _DMA_TYPES):
                    waits = list(si.on_wait)
                    extra = waits[max_waits:]
                    del si.on_wait[max_waits:]
                    for i in range(0, len(extra), max_waits):
                        _wsplit_counter[0] += 1
                        n_split += 1
                        newlist.append(mybir.InstNoOp(
                            name=f"I-wsplit-{_wsplit_counter[0]}",
                            engine=inst.engine, ins=[], outs=[],
                            sync_info=mybir.SyncInfo(
                                on_wait=extra[i:i + max_waits], on_update=[]),
                            text_hint="wsplit", bass_nofuse=True))
                newlist.append(inst)
            try:
                bb.instructions[:] = newlist
            except TypeError:
                bb.instructions = newlist
    return n_split


def _patched_drain_and_barrier(self, tick_clock, wait_clock):
    nc = self.nc
    drain_inst = nc.sync.drain()
    wait_clock.add_sem_waits(
        drain_inst.ins, ScopedClock({None: tick_clock.global_clock}))
    nc.all_engine_barrier()
    assert self.sems is not None
    popped = nc._tile_sem_poison_stack.pop()
    assert popped is self._sem_poison
    nc.clear_and_free_semaphores(list(self.sems.allocated().values()))
    nc.all_engine_barrier()


tile.TileContext._drain_and_barrier = _patched_drain_and_barrier


# ---------------------------------------------------------------------------
# Host-side preprocessing
# ---------------------------------------------------------------------------
def _bf(x):
    return np.ascontiguousarray(np.asarray(x, np.float32)).astype(BF16)


def _f32(x):
    return np.ascontiguousarray(np.asarray(x, np.float32))


def make_masks():
    i = np.arange(S)[:, None]
    j = np.arange(S)[None, :]
    d = np.abs(i - j)
    ms = []
    for h in range(5):
        dil = 2 ** h
        ms.append(((d <= dil) | (d % (2 ** (h + 1)) == 0)).astype(np.float32))
    return np.stack(ms)  # [5,S,S]


def preprocess(inp):
    w = {}
    c1 = np.asarray(inp["conv1_w"], np.float32)          # [256,64,3]
    w["c1w"] = _bf(c1.transpose(1, 2, 0).reshape(64, 3, 2, 128))
    c2 = np.asarray(inp["conv2_w"], np.float32)          # [512,256,3]
    # [p,dk,tap,mo,m] = conv2_w[mo*128+m, dk*128+p, tap]
    c2w = np.zeros((128, 2, 3, 4, 128), np.float32)
    for dk in range(2):
        for tap in range(3):
            for mo in range(4):
                c2w[:, dk, tap, mo, :] = c2[mo * 128:(mo + 1) * 128,
                                            dk * 128:(dk + 1) * 128, tap].T
    w["c2w"] = _bf(c2w)
    embw = np.asarray(inp["emb_w"], np.float32)          # [512,512]
    e4 = np.zeros((128, 4, 4, 128), np.float32)
    for dk in range(4):
        for mo in range(4):
            e4[:, dk, mo, :] = embw[mo * 128:(mo + 1) * 128,
                                    dk * 128:(dk + 1) * 128].T
    w["embw"] = _bf(e4)
    pos = np.asarray(inp["pos"], np.float32)[0]          # [S,D]
    w["post"] = _bf(pos.T.reshape(4, 128, 512).transpose(1, 0, 2))
    mk = make_masks()                                    # [5,S,S]
    w["masks"] = _bf(mk.reshape(5, 4, 128, 512).transpose(2, 0, 1, 3))
    # conv/emb biases: [128, 10] cols: c1b(2) c2b(4) embb(4)
    b0 = np.zeros((128, 10), np.float32)
    b0[:, 0:2] = _f32(inp["conv1_b"]).reshape(2, 128).T
    b0[:, 2:6] = _f32(inp["conv2_b"]).reshape(4, 128).T
    b0[:, 6:10] = _f32(inp["emb_b"]).reshape(4, 128).T
    w["bias0"] = b0

    wq_a = np.zeros((L, 128, 4, 4, 128), np.float32)
    wk_a = np.zeros((L, 128, 4, 4, 128), np.float32)
    wv_a = np.zeros((L, 128, 4, 512), np.float32)
    wo_a = np.zeros((L, 128, 4, 512), np.float32)
    w1_a = np.zeros((L, 128, 4, 2048), np.float32)
    w2_a = np.zeros((L, 128, 16, 512), np.float32)
    lb_a = np.zeros((L, 128, 32), np.float32)
    for l in range(L):
        g1 = np.asarray(inp["ln1_g"][l], np.float32)
        b1v = np.asarray(inp["ln1_b"][l], np.float32)
        g2 = np.asarray(inp["ln2_g"][l], np.float32)
        b2v = np.asarray(inp["ln2_b"][l], np.float32)
        Wq = np.asarray(inp["Wq"][l], np.float32)        # [H,E,D]
        Wk = np.asarray(inp["Wk"][l], np.float32)
        Wv = np.asarray(inp["Wv"][l], np.float32)
        Wo = np.asarray(inp["Wo"][l], np.float32)        # [D,D]
        bq = _f32(inp["bq"][l]) + np.einsum("hed,d->he", Wq, b1v)
        bk = _f32(inp["bk"][l]) + np.einsum("hed,d->he", Wk, b1v)
        bv = _f32(inp["bv"][l]) + np.einsum("hed,d->he", Wv, b1v)
        Wqg = Wq * g1[None, None, :]
        Wkg = Wk * g1[None, None, :]
        Wvg = Wv * g1[None, None, :]
        for pr in range(4):
            qT = np.concatenate([Wqg[2 * pr].T, Wqg[2 * pr + 1].T], 1)  # [D,128]
            kT = np.concatenate([Wkg[2 * pr].T, Wkg[2 * pr + 1].T], 1)
            for dk in range(4):
                wq_a[l, :, pr, dk, :] = qT[dk * 128:(dk + 1) * 128]
                wk_a[l, :, pr, dk, :] = kT[dk * 128:(dk + 1) * 128]
            lb_a[l, :, 0 + pr] = np.concatenate([bq[2 * pr], bq[2 * pr + 1]])
            lb_a[l, :, 4 + pr] = np.concatenate([bk[2 * pr], bk[2 * pr + 1]])
        vT = Wvg.transpose(2, 0, 1).reshape(D, H * E)    # [D, 512]
        woT = Wo.T                                       # [D_in, D_out]
        bo = _f32(inp["bo"][l]) + Wo @ bv.reshape(-1)
        W1g = np.asarray(inp["W1"][l], np.float32) * g2[None, :]
        b1f = _f32(inp["b1"][l]) + np.asarray(inp["W1"][l], np.float32) @ b2v
        W2 = np.asarray(inp["W2"][l], np.float32)
        for dk in range(4):
            wv_a[l, :, dk, :] = vT[dk * 128:(dk + 1) * 128]
            wo_a[l, :, dk, :] = woT[dk * 128:(dk + 1) * 128]
            w1_a[l, :, dk, :] = W1g.T[dk * 128:(dk + 1) * 128]
        for hk in range(16):
            w2_a[l, :, hk, :] = W2.T[hk * 128:(hk + 1) * 128]
        lb_a[l, :, 8:12] = bo.reshape(4, 128).T
        lb_a[l, :, 12:28] = b1f.reshape(16, 128).T
        lb_a[l, :, 28:32] = _f32(inp["b2"][l]).reshape(4, 128).T
    w["wq"], w["wk"], w["wv"], w["wo"] = map(_bf, (wq_a, wk_a, wv_a, wo_a))
    w["w1"], w["w2"] = _bf(w1_a), _bf(w2_a)
    w["lbias"] = lb_a

    # heads (lnf folded into layer 1; a3/c3 padded 57 -> 64)
    gf = np.asarray(inp["lnf_g"], np.float32)
    bfv = np.asarray(inp["lnf_b"], np.float32)
    hw1 = np.zeros((128, 2, 4, 128), np.float32)
    hw2 = np.zeros((128, 2, 2, 128), np.float32)
    hw3 = np.zeros((128, 2, 2, 64), np.float32)
    hw4 = np.zeros((64, 2, 3), np.float32)
    hb1 = np.zeros((128, 2), np.float32)
    hb2 = np.zeros((128, 2, 2), np.float32)
    hb3 = np.zeros((64, 2), np.float32)
    hb4 = np.zeros((3, 1), np.float32)
    hb4c = np.zeros((1, 1), np.float32)
    for ai, nm in enumerate(("a", "c")):
        w1h = np.asarray(inp[f"{nm}1_w"], np.float32) * gf[None, :]   # [128,512]
        b1h = _f32(inp[f"{nm}1_b"]) + np.asarray(inp[f"{nm}1_w"], np.float32) @ bfv
        for dk in range(4):
            hw1[:, ai, dk, :] = w1h.T[dk * 128:(dk + 1) * 128]
        hb1[:, ai] = b1h
        w2h = np.asarray(inp[f"{nm}2_w"], np.float32)                 # [256,128]
        for mo in range(2):
            hw2[:, ai, mo, :] = w2h[mo * 128:(mo + 1) * 128].T
        hb2[:, ai, :] = _f32(inp[f"{nm}2_b"]).reshape(2, 128).T
        w3h = np.asarray(inp[f"{nm}3_w"], np.float32)                 # [57,256]
        for dk in range(2):
            hw3[:, ai, dk, 0:57] = w3h[:, dk * 128:(dk + 1) * 128].T
        hb3[0:57, ai] = _f32(inp[f"{nm}3_b"])
        w4h = np.asarray(inp[f"{nm}4_w"], np.float32)                 # [A or 1, 57]
        if nm == "a":
            hw4[0:57, 0, :] = w4h.T
            hb4[:, 0] = _f32(inp["a4_b"])
        else:
            hw4[0:57, 1, 0:1] = w4h.T
            hb4c[:, 0] = _f32(inp["c4_b"])
    w["hw1"], w["hw2"], w["hw3"], w["hw4"] = map(_bf, (hw1, hw2, hw3, hw4))
    w["hb1"], w["hb2"], w["hb3"], w["hb4"], w["hb4c"] = hb1, hb2, hb3, hb4, hb4c
    return w


def prep_x(x):
    """x [B,S,F] -> per-core padded feature-major bf16 [64, BPC, 514]."""
    outs = []
    for c in range(NC):
        xp = np.zeros((F, BPC, S + 2), np.float32)
        xp[:, :, 1:S + 1] = np.asarray(
            x[c * BPC:(c + 1) * BPC], np.float32).transpose(2, 0, 1)
        outs.append(_bf(xp))
    return outs


# ---------------------------------------------------------------------------
# Bass kernel builder
# ---------------------------------------------------------------------------
def build_kernel(dbg_stage=""):
    nc = bass.Bass()

    def par(name, shape, dt=bf16):
        return nc.declare_dram_parameter(name, list(shape), dt, isOutput=False)

    xpad_d = par("xpad", [64, BPC, 514])
    c1w_d = par("c1w", [64, 3, 2, 128])
    c2w_d = par("c2w", [128, 2, 3, 4, 128])
    embw_d = par("embw", [128, 4, 4, 128])
    post_d = par("post", [128, 4, 512])
    masks_d = par("masks", [128, 5, 4, 512])
    bias0_d = par("bias0", [128, 10], f32)
    wq_d = par("wq", [L, 128, 4, 4, 128])
    wk_d = par("wk", [L, 128, 4, 4, 128])
    wv_d = par("wv", [L, 128, 4, 512])
    wo_d = par("wo", [L, 128, 4, 512])
    w1_d = par("w1", [L, 128, 4, 2048])
    w2_d = par("w2", [L, 128, 16, 512])
    lbias_d = par("lbias", [L, 128, 32], f32)
    hw1_d = par("hw1", [128, 2, 4, 128])
    hw2_d = par("hw2", [128, 2, 2, 128])
    hw3_d = par("hw3", [128, 2, 2, 64])
    hw4_d = par("hw4", [64, 2, 3])
    hb1_d = par("hb1", [128, 2], f32)
    hb2_d = par("hb2", [128, 2, 2], f32)
    hb3_d = par("hb3", [64, 2], f32)
    hb4_d = par("hb4", [3, 1], f32)
    hb4c_d = par("hb4c", [1, 1], f32)
    out_d = nc.declare_dram_parameter("out", [4, BPC], f32, isOutput=True)
    dbg_d = None
    if dbg_stage:
        dbg_d = nc.declare_dram_parameter("dbg", [128, 4, N], f32, isOutput=True)

    with tile.TileContext(nc) as tc:
        with tc.tile_pool(name="single", bufs=1) as single, \
             tc.tile_pool(name="wpool", bufs=2) as wpool, \
             tc.tile_pool(name="big1", bufs=1) as big1, \
             tc.tile_pool(name="per2", bufs=2) as per2, \
             tc.tile_pool(name="per3", bufs=3) as per3, \
             tc.tile_pool(name="ps", bufs=1, space="PSUM") as ps:

            # ---------------- persistent tiles ----------------
            x_sb = single.tile([64, BPC, 514], bf16)
            nc.sync.dma_start(out=x_sb, in_=xpad_d[:])
            c1w_t = single.tile([64, 3, 2, 128], bf16)
            nc.sync.dma_start(out=c1w_t, in_=c1w_d[:])
            c2w_t = single.tile([128, 2, 3, 4, 128], bf16)
            nc.sync.dma_start(out=c2w_t, in_=c2w_d[:])
            embw_t = single.tile([128, 4, 4, 128], bf16)
            nc.sync.dma_start(out=embw_t, in_=embw_d[:])
            pos_t = single.tile([128, 4, 512], bf16)
            nc.sync.dma_start(out=pos_t, in_=post_d[:])
            mask_t = single.tile([128, 5, 4, 512], bf16)
            nc.sync.dma_start(out=mask_t, in_=masks_d[:])
            bias0_t = single.tile([128, 10], f32)
            nc.sync.dma_start(out=bias0_t, in_=bias0_d[:])

            ones128 = single.tile([128, 1], bf16)   # stats lhsT
            nc.vector.memset(ones128, 1.0)
            ones1 = single.tile([1, 128], bf16)     # bcast lhsT
            nc.vector.memset(ones1, 1.0)
            nones = single.tile([1, 128], bf16)     # bcast lhsT * (-1/512)
            nc.vector.memset(nones, -1.0 / 512.0)
            eps1 = single.tile([1, 1], f32)
            nc.vector.memset(eps1, 1e-5)

            zt = big1.tile([128, 4, N], f32)        # residual stream
            h1_sb = big1.tile([128, 2, BPC, 514], bf16)
            nc.vector.memset(h1_sb, 0.0)
            h2_sb = big1.tile([128, 4, BPC, 512], bf16)

            # ---------------- conv feature extractor ----------------
            for s in range(BPC):
                for mc in range(2):
                    pc = ps.tile([128, 512], f32, tag="mm", bufs=2)
                    for d in range(3):
                        nc.tensor.matmul(pc, c1w_t[:, d, mc, :],
                                         x_sb[:, s, d:d + 512],
                                         start=(d == 0), stop=(d == 2))
                    nc.scalar.activation(out=h1_sb[:, mc, s, 1:513], in_=pc,
                                         func=AF.Gelu, bias=bias0_t[:, mc:mc + 1])
            for s in range(BPC):
                for mo in range(4):
                    pc = ps.tile([128, 512], f32, tag="mm", bufs=2)
                    k = 0
                    for dk in range(2):
                        for d in range(3):
                            nc.tensor.matmul(pc, c2w_t[:, dk, d, mo, :],
                                             h1_sb[:, dk, s, d:d + 512],
                                             start=(k == 0), stop=(k == 5))
                            k += 1
                    nc.scalar.activation(out=h2_sb[:, mo, s, :], in_=pc,
                                         func=AF.Gelu,
                                         bias=bias0_t[:, 2 + mo:3 + mo])
            # ---------------- input embedding ----------------
            for s in range(BPC):
                for mo in range(4):
                    pc = ps.tile([128, 512], f32, tag="mm", bufs=2)
                    for dk in range(4):
                        nc.tensor.matmul(pc, embw_t[:, dk, mo, :],
                                         h2_sb[:, dk, s, :],
                                         start=(dk == 0), stop=(dk == 3))
                    nc.vector.scalar_tensor_tensor(
                        out=zt[:, mo, s * S:(s + 1) * S], in0=pc,
                        scalar=bias0_t[:, 6 + mo:7 + mo], in1=pos_t[:, mo, :],
                        op0=OP.add, op1=OP.add)

            # ---------------- LN helper ----------------
            def layer_norm(dst_u, s):
                """LN over feature dim of zt columns of sample s -> dst_u
                (bf16 [128,4,N]); stats via ones-matmuls; rstd via ln/exp."""
                sl = slice(s * S, (s + 1) * S)
                zbf = per2.tile([128, 4, S], bf16, tag="zbf", bufs=1)
                zsq = per2.tile([128, 4, S], bf16, tag="zsq", bufs=1)
                for c in range(4):
                    nc.gpsimd.tensor_copy(out=zbf[:, c, :], in_=zt[:, c, sl])
                    nc.scalar.activation(out=zsq[:, c, :], in_=zt[:, c, sl],
                                         func=AF.Square)
                p_sz = ps.tile([1, 512], f32, tag="st", bufs=2)
                p_sq = ps.tile([1, 512], f32, tag="st", bufs=2)
                for c in range(4):
                    nc.tensor.matmul(p_sz, ones128, zbf[:, c, :],
                                     start=(c == 0), stop=(c == 3))
                for c in range(4):
                    nc.tensor.matmul(p_sq, ones128, zsq[:, c, :],
                                     start=(c == 0), stop=(c == 3))
                sumz = per3.tile([1, 512], f32, tag="sumz", bufs=2)
                nc.vector.tensor_copy(out=sumz, in_=p_sz)
                t2 = per3.tile([1, 512], f32, tag="tchain", bufs=4, name="t2")
                nc.vector.tensor_scalar(out=t2, in0=p_sq, scalar1=512.0,
                                        scalar2=None, op0=OP.mult)
                t1 = per3.tile([1, 512], f32, tag="tchain", bufs=4, name="t1")
                nc.vector.tensor_tensor(out=t1, in0=sumz, in1=sumz, op=OP.mult)
                vr = per3.tile([1, 512], f32, tag="tchain", bufs=4, name="vr")
                nc.vector.tensor_tensor(out=vr, in0=t2, in1=t1, op=OP.subtract)
                lnv = per3.tile([1, 512], f32, tag="tchain", bufs=4, name="lnv")
                nc.scalar.activation(out=lnv, in_=vr, func=AF.Ln,
                                     scale=1.0 / (512.0 * 512.0), bias=eps1)
                s1row = per3.tile([1, 512], bf16, tag="srow", bufs=4, name="s1row")
                nc.scalar.activation(out=s1row, in_=lnv, func=AF.Exp, scale=-0.5)
                s2tmp = per3.tile([1, 512], bf16, tag="srow", bufs=4, name="s2tmp")
                nc.vector.tensor_tensor(out=s2tmp, in0=sumz, in1=s1row,
                                        op=OP.mult)
                p_s1b = ps.tile([128, 512], f32, tag="st", bufs=2)
                nc.tensor.matmul(p_s1b, ones1, s1row, start=True, stop=True)
                p_s2b = ps.tile([128, 512], f32, tag="st", bufs=2)
                nc.tensor.matmul(p_s2b, nones, s2tmp, start=True, stop=True)
                for c in range(4):
                    nc.vector.tensor_tensor(out=dst_u[:, c, sl],
                                            in0=zt[:, c, sl],
                                            in1=p_s1b, op=OP.mult)
                    nc.vector.tensor_tensor(out=dst_u[:, c, sl],
                                            in0=dst_u[:, c, sl],
                                            in1=p_s2b, op=OP.add)

            # ---------------- transformer layers ----------------
            for l in range(L):
                wq_t = wpool.tile([128, 4, 4, 128], bf16, tag="wq", bufs=1)
                nc.sync.dma_start(out=wq_t, in_=wq_d[l])
                wk_t = wpool.tile([128, 4, 4, 128], bf16, tag="wk", bufs=1)
                nc.sync.dma_start(out=wk_t, in_=wk_d[l])
                wv_t = wpool.tile([128, 4, 512], bf16, tag="wv", bufs=1)
                nc.sync.dma_start(out=wv_t, in_=wv_d[l])
                wo_t = wpool.tile([128, 4, 512], bf16, tag="wo", bufs=1)
                nc.sync.dma_start(out=wo_t, in_=wo_d[l])
                w1_t = wpool.tile([128, 4, 2048], bf16, tag="w1", bufs=1)
                nc.sync.dma_start(out=w1_t, in_=w1_d[l])
                w2_t = wpool.tile([128, 16, 512], bf16, tag="w2", bufs=1)
                nc.sync.dma_start(out=w2_t, in_=w2_d[l])
                lb_t = wpool.tile([128, 32], f32, tag="lb", bufs=1)
                nc.sync.dma_start(out=lb_t, in_=lbias_d[l])

                u = per2.tile([128, 4, N], bf16, tag="u", bufs=1)
                for s in range(BPC):
                    layer_norm(u, s)

                # QKV projections
                q_all = per2.tile([128, 4, N], bf16, tag="q", bufs=1)
                k_all = per2.tile([128, 4, N], bf16, tag="k", bufs=1)
                for dst, wt, bcol in ((q_all, wq_t, 0), (k_all, wk_t, 4)):
                    for pr in range(4):
                        for half in range(2):
                            pc = ps.tile([128, 512], f32, tag="mm", bufs=2)
                            for dk in range(4):
                                nc.tensor.matmul(
                                    pc, wt[:, pr, dk, :],
                                    u[:, dk, half * 512:(half + 1) * 512],
                                    start=(dk == 0), stop=(dk == 3))
                            nc.scalar.activation(
                                out=dst[:, pr, half * 512:(half + 1) * 512],
                                in_=pc, func=AF.Identity,
                                bias=lb_t[:, bcol + pr:bcol + pr + 1])
                v_aug = per2.tile([128, 8, 8, 65], bf16, tag="vaug", bufs=1)
                nc.vector.memset(v_aug[:, :, :, 64:65], 1.0)
                for tcg in range(8):
                    pc = ps.tile([128, 512], f32, tag="mm", bufs=2)
                    for dk in range(4):
                        nc.tensor.matmul(
                            pc, u[:, dk, tcg * 128:(tcg + 1) * 128],
                            wv_t[:, dk, :], start=(dk == 0), stop=(dk == 3))
                    for h in range(H):
                        nc.vector.tensor_copy(
                            out=v_aug[:, h, tcg, 0:64],
                            in_=pc[:, h * 64:(h + 1) * 64])

                # attention
                o_all = per2.tile([128, 4, N], bf16, tag="oall", bufs=1)
                for s in range(BPC):
                    for pr in range(4):
                        rts = [per3.tile([1, 512], f32, tag="rt0", name="rt0", bufs=2),
                               per3.tile([1, 512], f32, tag="rt1", name="rt1", bufs=2)]
                        rtbs = [per3.tile([1, 512], bf16, tag="rtb0", name="rtb0", bufs=2),
                                per3.tile([1, 512], bf16, tag="rtb1", name="rtb1", bufs=2)]
                        # 1/r via exp(-ln(r)) on ScalarE (stays in ln/exp
                        # table set; custom-DVE recip is broken in this
                        # toolchain and vector.reciprocal costs 3.4us)
                        pvs = []
                        for o in range(2):
                            h = 2 * pr + o
                            ob = o * 64
                            pv = ps.tile([65, 512], f32, tag="pv", bufs=4)
                            pvs.append(pv)
                            for tcl in range(4):
                                pss = ps.tile([128, 512], f32, tag="mm", bufs=2)
                                tg = s * 512 + tcl * 128
                                nc.tensor.matmul(
                                    pss,
                                    k_all[ob:ob + 64, pr, tg:tg + 128],
                                    q_all[ob:ob + 64, pr, s * 512:(s + 1) * 512],
                                    start=True, stop=True)
                                pt = per3.tile([128, 512], bf16, tag="pt")
                                nc.scalar.activation(out=pt, in_=pss,
                                                     func=AF.Exp, scale=0.125)
                                nc.vector.tensor_tensor(
                                    out=pt, in0=pt,
                                    in1=mask_t[:, min(h, 4), tcl, :],
                                    op=OP.mult)
                                nc.tensor.matmul(pv, v_aug[:, h, s * 4 + tcl, :],
                                                 pt, start=(tcl == 0),
                                                 stop=(tcl == 3))
                            nc.scalar.activation(out=rts[o], in_=pv[64:65, :],
                                                 func=AF.Ln)
                            nc.scalar.activation(out=rtbs[o], in_=rts[o],
                                                 func=AF.Exp, scale=-1.0)
                        p_b = ps.tile([128, 512], f32, tag="st", bufs=2)
                        nc.tensor.matmul(p_b[0:64, :], ones1[:, 0:64],
                                         rtbs[0], start=True, stop=True)
                        nc.tensor.matmul(p_b[64:128, :], ones1[:, 0:64],
                                         rtbs[1], start=True, stop=True)
                        b_sb = per3.tile([128, 512], bf16, tag="bsb")
                        nc.scalar.activation(out=b_sb, in_=p_b, func=AF.Identity)
                        for o in range(2):
                            nc.vector.tensor_tensor(
                                out=o_all[o * 64:(o + 1) * 64, pr,
                                          s * 512:(s + 1) * 512],
                                in0=pvs[o][0:64, :],
                                in1=b_sb[o * 64:(o + 1) * 64, :], op=OP.mult)

                # output projection + residual
                for s in range(BPC):
                    for mo in range(4):
                        pc = ps.tile([128, 512], f32, tag="mm", bufs=2)
                        for dk in range(4):
                            nc.tensor.matmul(
                                pc, wo_t[:, dk, mo * 128:(mo + 1) * 128],
                                o_all[:, dk, s * 512:(s + 1) * 512],
                                start=(dk == 0), stop=(dk == 3))
                        nc.vector.scalar_tensor_tensor(
                            out=zt[:, mo, s * S:(s + 1) * S], in0=pc,
                            scalar=lb_t[:, 8 + mo:9 + mo],
                            in1=zt[:, mo, s * S:(s + 1) * S],
                            op0=OP.add, op1=OP.add)

                # FFN
                u2 = per2.tile([128, 4, N], bf16, tag="u", bufs=1)
                for s in range(BPC):
                    layer_norm(u2, s)
                for s in range(BPC):
                    pas = [ps.tile([128, 512], f32, tag="pv", bufs=4,
                                   name=f"pa{mo}") for mo in range(4)]
                    for hc in range(16):
                        pc = ps.tile([128, 512], f32, tag="mm", bufs=2)
                        for dk in range(4):
                            nc.tensor.matmul(
                                pc, w1_t[:, dk, hc * 128:(hc + 1) * 128],
                                u2[:, dk, s * 512:(s + 1) * 512],
                                start=(dk == 0), stop=(dk == 3))
                        hgel = per3.tile([128, 512], bf16, tag="hgel")
                        nc.scalar.activation(out=hgel, in_=pc,
                                             func=AF.Gelu,
                                             bias=lb_t[:, 12 + hc:13 + hc])
                        for mo in range(4):
                            nc.tensor.matmul(
                                pas[mo], w2_t[:, hc, mo * 128:(mo + 1) * 128],
                                hgel, start=(hc == 0), stop=(hc == 15))
                    for mo in range(4):
                        nc.vector.scalar_tensor_tensor(
                            out=zt[:, mo, s * S:(s + 1) * S], in0=pas[mo],
                            scalar=lb_t[:, 28 + mo:29 + mo],
                            in1=zt[:, mo, s * S:(s + 1) * S],
                            op0=OP.add, op1=OP.add)
                if dbg_stage == f"l{l}":
                    dg = per2.tile([128, 4, N], f32, tag="dbg")
                    for c in range(4):
                        nc.vector.tensor_copy(out=dg[:, c, :], in_=zt[:, c, :])
                    nc.sync.dma_start(out=dbg_d[:], in_=dg)
            if dbg_stage == "emb":
                dg = per2.tile([128, 4, N], f32, tag="dbg")
                for c in range(4):
                    nc.vector.tensor_copy(out=dg[:, c, :], in_=zt[:, c, :])
                nc.sync.dma_start(out=dbg_d[:], in_=dg)

            # ---------------- final LN on last-token columns ----------------
            hw1_t = single.tile([128, 2, 4, 128], bf16)
            nc.sync.dma_start(out=hw1_t, in_=hw1_d[:])
            hw2_t = single.tile([128, 2, 2, 128], bf16)
            nc.sync.dma_start(out=hw2_t, in_=hw2_d[:])
            hw3_t = single.tile([128, 2, 2, 64], bf16)
            nc.sync.dma_start(out=hw3_t, in_=hw3_d[:])
            hw4_t = single.tile([64, 2, 3], bf16)
            nc.sync.dma_start(out=hw4_t, in_=hw4_d[:])
            hb1_t = single.tile([128, 2], f32)
            nc.sync.dma_start(out=hb1_t, in_=hb1_d[:])
            hb2_t = single.tile([128, 2, 2], f32)
            nc.sync.dma_start(out=hb2_t, in_=hb2_d[:])
            hb3_t = single.tile([64, 2], f32)
            nc.sync.dma_start(out=hb3_t, in_=hb3_d[:])
            hb4_t = single.tile([3, 1], f32)
            nc.sync.dma_start(out=hb4_t, in_=hb4_d[:])
            hb4c_t = single.tile([1, 1], f32)
            nc.sync.dma_start(out=hb4c_t, in_=hb4c_d[:])

            zf_b = single.tile([128, 4, BPC], bf16)
            zf_q = single.tile([128, 4, BPC], bf16)
            for c in range(4):
                nc.vector.tensor_copy(out=zf_b[:, c, :],
                                      in_=zt[:, c, S - 1:N:S])
                nc.scalar.activation(out=zf_q[:, c, :], in_=zt[:, c, S - 1:N:S],
                                     func=AF.Square)
            pf_sz = ps.tile([1, BPC], f32, tag="st", bufs=2)
            pf_sq = ps.tile([1, BPC], f32, tag="st", bufs=2)
            for c in range(4):
                nc.tensor.matmul(pf_sz, ones128, zf_b[:, c, :],
                                 start=(c == 0), stop=(c == 3))
            for c in range(4):
                nc.tensor.matmul(pf_sq, ones128, zf_q[:, c, :],
                                 start=(c == 0), stop=(c == 3))
            sumzf = single.tile([1, BPC], f32)
            nc.vector.tensor_copy(out=sumzf, in_=pf_sz)
            t2f = single.tile([1, BPC], f32)
            nc.vector.tensor_scalar(out=t2f, in0=pf_sq, scalar1=512.0,
                                    scalar2=None, op0=OP.mult)
            t1f = single.tile([1, BPC], f32)
            nc.vector.tensor_tensor(out=t1f, in0=sumzf, in1=sumzf, op=OP.mult)
            vrf = single.tile([1, BPC], f32)
            nc.vector.tensor_tensor(out=vrf, in0=t2f, in1=t1f, op=OP.subtract)
            lnvf = single.tile([1, BPC], f32)
            nc.scalar.activation(out=lnvf, in_=vrf, func=AF.Ln,
                                 scale=1.0 / (512.0 * 512.0), bias=eps1)
            s1f = single.tile([1, BPC], bf16)
            nc.scalar.activation(out=s1f, in_=lnvf, func=AF.Exp, scale=-0.5)
            s2f = single.tile([1, BPC], bf16)
            nc.vector.tensor_tensor(out=s2f, in0=sumzf, in1=s1f, op=OP.mult)
            pf_s1 = ps.tile([128, BPC], f32, tag="st", bufs=2)
            nc.tensor.matmul(pf_s1, ones1, s1f, start=True, stop=True)
            pf_s2 = ps.tile([128, BPC], f32, tag="st", bufs=2)
            nc.tensor.matmul(pf_s2, nones, s2f, start=True, stop=True)
            feat = single.tile([128, 4, BPC], bf16)
            for c in range(4):
                tmpf = single.tile([128, BPC], f32, tag=f"tmpf{c}")
                nc.vector.tensor_tensor(out=tmpf, in0=zt[:, c, S - 1:N:S],
                                        in1=pf_s1, op=OP.mult)
                nc.vector.tensor_tensor(out=feat[:, c, :], in0=tmpf,
                                        in1=pf_s2, op=OP.add)

            # ---------------- actor/critic heads ----------------
            outs = []
            for ai in range(2):
                pc = ps.tile([128, BPC], f32, tag="st", bufs=2)
                for dk in range(4):
                    nc.tensor.matmul(pc, hw1_t[:, ai, dk, :], feat[:, dk, :],
                                     start=(dk == 0), stop=(dk == 3))
                f1 = single.tile([128, BPC], bf16, tag=f"f1_{ai}")
                nc.scalar.activation(out=f1, in_=pc, func=AF.Gelu,
                                     bias=hb1_t[:, ai:ai + 1])
                f2 = single.tile([128, 2, BPC], bf16, tag=f"f2_{ai}")
                for mo in range(2):
                    pc2 = ps.tile([128, BPC], f32, tag="st", bufs=2)
                    nc.tensor.matmul(pc2, hw2_t[:, ai, mo, :], f1,
                                     start=True, stop=True)
                    nc.scalar.activation(out=f2[:, mo, :], in_=pc2, func=AF.Gelu,
                                         bias=hb2_t[:, ai, mo:mo + 1])
                pc3 = ps.tile([64, BPC], f32, tag="st", bufs=2)
                for dk in range(2):
                    nc.tensor.matmul(pc3, hw3_t[:, ai, dk, :], f2[:, dk, :],
                                     start=(dk == 0), stop=(dk == 1))
                f3 = single.tile([64, BPC], bf16, tag=f"f3_{ai}")
                nc.scalar.activation(out=f3, in_=pc3, func=AF.Gelu,
                                     bias=hb3_t[:, ai:ai + 1])
                outs.append(f3)
            pol_ps = ps.tile([3, BPC], f32, tag="st", bufs=2)
            nc.tensor.matmul(pol_ps, hw4_t[:, 0, :], outs[0],
                             start=True, stop=True)
            pol_sb = single.tile([3, BPC], f32)
            nc.scalar.activation(out=pol_sb, in_=pol_ps, func=AF.Identity,
                                 bias=hb4_t[:, 0:1])
            val_ps = ps.tile([1, BPC], f32, tag="st", bufs=2)
            nc.tensor.matmul(val_ps, hw4_t[:, 1, 0:1], outs[1],
                             start=True, stop=True)
            val_sb = single.tile([1, BPC], f32)
            nc.scalar.activation(out=val_sb, in_=val_ps, func=AF.Identity,
                                 bias=hb4c_t[:, 0:1])
            nc.sync.dma_start(out=out_d[0:3, :], in_=pol_sb)
            nc.sync.dma_start(out=out_d[3:4, :], in_=val_sb)

    n = split_sync_waits(nc)
    return nc


# ---------------------------------------------------------------------------
# Entry point
# ---------------------------------------------------------------------------
_CACHE = {}


def kernel(**inputs):
    dbg_stage = os.environ.get("BASS_DBG_STAGE", "")
    key = ("nc", dbg_stage)
    if key not in _CACHE:
        _CACHE[key] = build_kernel(dbg_stage)
    nc = _CACHE[key]
    w = preprocess(inputs)
    xs = prep_x(np.asarray(inputs["x"], np.float32))
    in_maps = []
    for c in range(NC):
        m = {"xpad": xs[c]}
        m.update(w)
        in_maps.append(m)
    trace = os.environ.get("BASS_KERNEL_TRACE", "") == "1"
    res = run_bass_kernel_spmd(nc, in_maps, core_ids=list(range(NC)),
                               trace=trace)
    kernel.last_result = res
    policy = np.zeros((B, A), np.float32)
    value = np.zeros((B, 1), np.float32)
    for c in range(NC):
        o = np.asarray(res.results[c]["out"], np.float32)
        policy[c * BPC:(c + 1) * BPC] = o[0:3].T
        value[c * BPC:(c + 1) * BPC] = o[3:4].T
    if dbg_stage:
        kernel.dbg = [np.asarray(res.results[c]["dbg"]) for c in range(NC)]
    return policy, value


# revision 11
# speedup vs baseline: 1.1171x; 1.0247x over previous
"""Trainium2 Bass kernel for nn_ActorCriticTransformer (sparse_attention).

Strategy: pure data-parallel over batch (16 samples / 8 cores = 2 per core).
Per-core: conv feature extractor + 4 PyraFormer blocks + actor/critic heads,
computed feature-major ([D partitions, tokens free]) in bf16 with f32 PSUM
accumulation and an f32 residual stream.

Key algebraic tricks (validated in numpy golden sim):
 - LayerNorm gamma/beta folded into adjacent projection weights host-side;
   on-chip LN reduces to stats + broadcast + 2 DVE ops.
 - Attention computed in transposed-score layout  sT[t,s] = K @ Q^T  so no
   PE transposes are needed anywhere; V is produced token-major directly by
   swapping matmul operand roles; a ones-column appended to V yields softmax
   row-sums for free; softmax normalization deferred to a column-scaling of
   the per-head output (commutes with Wo).
 - Softmax computed without max-subtraction (inputs are bounded); sparse
   masks applied multiplicatively post-exp ({0,1} masks resident in SBUF).
"""
import os
import numpy as np
import ml_dtypes

import concourse.bass as bass
import concourse.mybir as mybir
import concourse.tile as tile
from concourse.vector_clock import ScopedClock
from concourse.bass_utils import run_bass_kernel_spmd

BF16 = ml_dtypes.bfloat16
f32 = mybir.dt.float32
bf16 = mybir.dt.bfloat16
AF = mybir.ActivationFunctionType
OP = mybir.AluOpType

B, S, F, D, H, L, A = 16, 512, 64, 512, 8, 4, 3
E = D // H
NC = 8
BPC = B // NC
N = BPC * S
P = 128

# ---------------------------------------------------------------------------
# walrus in this toolchain supports only ONE sync wait per instruction.
# Patch the Tile tail drain and add a global post-pass splitting extra waits
# onto fresh same-engine nops.
# ---------------------------------------------------------------------------
_DMA_TYPES = tuple(
    t for t in (
        getattr(mybir, "InstTensorLoad", None),
        getattr(mybir, "InstTensorSave", None),
        getattr(mybir, "InstDmaTrigger", None),
        getattr(mybir, "InstTensorLoadIndirect", None),
        getattr(mybir, "InstTensorSaveIndirect", None),
    ) if t is not None
)
_wsplit_counter = [0]


def split_sync_waits(nc, max_waits=1):
    n_split = 0
    for fn in nc.m.functions:
        for bb in fn.blocks:
            if not any(
                i.sync_info is not None and i.sync_info.on_wait
                and len(i.sync_info.on_wait) > max_waits
                and not isinstance(i, _DMA_TYPES)
                for i in bb.instructions
            ):
                continue
            newlist = []
            for inst in bb.instructions:
                si = inst.sync_info
                if si is not None and si.on_wait and len(si.on_wait) > max_waits \
                        and not isinstance(inst, _DMA_TYPES):
                    waits = list(si.on_wait)
                    extra = waits[max_waits:]
                    del si.on_wait[max_waits:]
                    for i in range(0, len(extra), max_waits):
                        _wsplit_counter[0] += 1
                        n_split += 1
                        newlist.append(mybir.InstNoOp(
                            name=f"I-wsplit-{_wsplit_counter[0]}",
                            engine=inst.engine, ins=[], outs=[],
                            sync_info=mybir.SyncInfo(
                                on_wait=extra[i:i + max_waits], on_update=[]),
                            text_hint="wsplit", bass_nofuse=True))
                newlist.append(inst)
            try:
                bb.instructions[:] = newlist
            except TypeError:
                bb.instructions = newlist
    return n_split


def _patched_drain_and_barrier(self, tick_clock, wait_clock):
    nc = self.nc
    drain_inst = nc.sync.drain()
    wait_clock.add_sem_waits(
        drain_inst.ins, ScopedClock({None: tick_clock.global_clock}))
    nc.all_engine_barrier()
    assert self.sems is not None
    popped = nc._tile_sem_poison_stack.pop()
    assert popped is self._sem_poison
    nc.clear_and_free_semaphores(list(self.sems.allocated().values()))
    nc.all_engine_barrier()


tile.TileContext._drain_and_barrier = _patched_drain_and_barrier


# ---------------------------------------------------------------------------
# Host-side preprocessing
# ---------------------------------------------------------------------------
def _bf(x):
    return np.ascontiguousarray(np.asarray(x, np.float32)).astype(BF16)


def _f32(x):
    return np.ascontiguousarray(np.asarray(x, np.float32))


def make_masks():
    i = np.arange(S)[:, None]
    j = np.arange(S)[None, :]
    d = np.abs(i - j)
    ms = []
    for h in range(5):
        dil = 2 ** h
        ms.append(((d <= dil) | (d % (2 ** (h + 1)) == 0)).astype(np.float32))
    return np.stack(ms)  # [5,S,S]


def preprocess(inp):
    w = {}
    c1 = np.asarray(inp["conv1_w"], np.float32)          # [256,64,3]
    w["c1w"] = _bf(c1.transpose(1, 2, 0).reshape(64, 3, 2, 128))
    c2 = np.asarray(inp["conv2_w"], np.float32)          # [512,256,3]
    # [p,dk,tap,mo,m] = conv2_w[mo*128+m, dk*128+p, tap]
    c2w = np.zeros((128, 2, 3, 4, 128), np.float32)
    for dk in range(2):
        for tap in range(3):
            for mo in range(4):
                c2w[:, dk, tap, mo, :] = c2[mo * 128:(mo + 1) * 128,
                                            dk * 128:(dk + 1) * 128, tap].T
    w["c2w"] = _bf(c2w)
    embw = np.asarray(inp["emb_w"], np.float32)          # [512,512]
    e4 = np.zeros((128, 4, 4, 128), np.float32)
    for dk in range(4):
        for mo in range(4):
            e4[:, dk, mo, :] = embw[mo * 128:(mo + 1) * 128,
                                    dk * 128:(dk + 1) * 128].T
    w["embw"] = _bf(e4)
    pos = np.asarray(inp["pos"], np.float32)[0]          # [S,D]
    w["post"] = _bf(pos.T.reshape(4, 128, 512).transpose(1, 0, 2))
    mk = make_masks()                                    # [5,S,S]
    w["masks"] = _bf(mk.reshape(5, 4, 128, 512).transpose(2, 0, 1, 3))
    # conv/emb biases: [128, 10] cols: c1b(2) c2b(4) embb(4)
    b0 = np.zeros((128, 10), np.float32)
    b0[:, 0:2] = _f32(inp["conv1_b"]).reshape(2, 128).T
    b0[:, 2:6] = _f32(inp["conv2_b"]).reshape(4, 128).T
    b0[:, 6:10] = _f32(inp["emb_b"]).reshape(4, 128).T
    w["bias0"] = b0

    wq_a = np.zeros((L, 128, 4, 4, 128), np.float32)
    wk_a = np.zeros((L, 128, 4, 4, 128), np.float32)
    wv_a = np.zeros((L, 128, 4, 512), np.float32)
    wo_a = np.zeros((L, 128, 4, 512), np.float32)
    w1_a = np.zeros((L, 128, 4, 2048), np.float32)
    w2_a = np.zeros((L, 128, 16, 512), np.float32)
    lb_a = np.zeros((L, 128, 32), np.float32)
    for l in range(L):
        g1 = np.asarray(inp["ln1_g"][l], np.float32)
        b1v = np.asarray(inp["ln1_b"][l], np.float32)
        g2 = np.asarray(inp["ln2_g"][l], np.float32)
        b2v = np.asarray(inp["ln2_b"][l], np.float32)
        Wq = np.asarray(inp["Wq"][l], np.float32)        # [H,E,D]
        Wk = np.asarray(inp["Wk"][l], np.float32)
        Wv = np.asarray(inp["Wv"][l], np.float32)
        Wo = np.asarray(inp["Wo"][l], np.float32)        # [D,D]
        bq = _f32(inp["bq"][l]) + np.einsum("hed,d->he", Wq, b1v)
        bk = _f32(inp["bk"][l]) + np.einsum("hed,d->he", Wk, b1v)
        bv = _f32(inp["bv"][l]) + np.einsum("hed,d->he", Wv, b1v)
        Wqg = Wq * g1[None, None, :]
        Wkg = Wk * g1[None, None, :]
        Wvg = Wv * g1[None, None, :]
        for pr in range(4):
            qT = np.concatenate([Wqg[2 * pr].T, Wqg[2 * pr + 1].T], 1)  # [D,128]
            kT = np.concatenate([Wkg[2 * pr].T, Wkg[2 * pr + 1].T], 1)
            for dk in range(4):
                wq_a[l, :, pr, dk, :] = qT[dk * 128:(dk + 1) * 128]
                wk_a[l, :, pr, dk, :] = kT[dk * 128:(dk + 1) * 128]
            lb_a[l, :, 0 + pr] = np.concatenate([bq[2 * pr], bq[2 * pr + 1]])
            lb_a[l, :, 4 + pr] = np.concatenate([bk[2 * pr], bk[2 * pr + 1]])
        vT = Wvg.transpose(2, 0, 1).reshape(D, H * E)    # [D, 512]
        woT = Wo.T                                       # [D_in, D_out]
        bo = _f32(inp["bo"][l]) + Wo @ bv.reshape(-1)
        W1g = np.asarray(inp["W1"][l], np.float32) * g2[None, :]
        b1f = _f32(inp["b1"][l]) + np.asarray(inp["W1"][l], np.float32) @ b2v
        W2 = np.asarray(inp["W2"][l], np.float32)
        for dk in range(4):
            wv_a[l, :, dk, :] = vT[dk * 128:(dk + 1) * 128]
            wo_a[l, :, dk, :] = woT[dk * 128:(dk + 1) * 128]
            w1_a[l, :, dk, :] = W1g.T[dk * 128:(dk + 1) * 128]
        for hk in range(16):
            w2_a[l, :, hk, :] = W2.T[hk * 128:(hk + 1) * 128]
        lb_a[l, :, 8:12] = bo.reshape(4, 128).T
        lb_a[l, :, 12:28] = b1f.reshape(16, 128).T
        lb_a[l, :, 28:32] = _f32(inp["b2"][l]).reshape(4, 128).T
    w["wq"], w["wk"], w["wv"], w["wo"] = map(_bf, (wq_a, wk_a, wv_a, wo_a))
    w["w1"], w["w2"] = _bf(w1_a), _bf(w2_a)
    w["lbias"] = lb_a

    # heads (lnf folded into layer 1; a3/c3 padded 57 -> 64)
    gf = np.asarray(inp["lnf_g"], np.float32)
    bfv = np.asarray(inp["lnf_b"], np.float32)
    hw1 = np.zeros((128, 2, 4, 128), np.float32)
    hw2 = np.zeros((128, 2, 2, 128), np.float32)
    hw3 = np.zeros((128, 2, 2, 64), np.float32)
    hw4 = np.zeros((64, 2, 3), np.float32)
    hb1 = np.zeros((128, 2), np.float32)
    hb2 = np.zeros((128, 2, 2), np.float32)
    hb3 = np.zeros((64, 2), np.float32)
    hb4 = np.zeros((3, 1), np.float32)
    hb4c = np.zeros((1, 1), np.float32)
    for ai, nm in enumerate(("a", "c")):
        w1h = np.asarray(inp[f"{nm}1_w"], np.float32) * gf[None, :]   # [128,512]
        b1h = _f32(inp[f"{nm}1_b"]) + np.asarray(inp[f"{nm}1_w"], np.float32) @ bfv
        for dk in range(4):
            hw1[:, ai, dk, :] = w1h.T[dk * 128:(dk + 1) * 128]
        hb1[:, ai] = b1h
        w2h = np.asarray(inp[f"{nm}2_w"], np.float32)                 # [256,128]
        for mo in range(2):
            hw2[:, ai, mo, :] = w2h[mo * 128:(mo + 1) * 128].T
        hb2[:, ai, :] = _f32(inp[f"{nm}2_b"]).reshape(2, 128).T
        w3h = np.asarray(inp[f"{nm}3_w"], np.float32)                 # [57,256]
        for dk in range(2):
            hw3[:, ai, dk, 0:57] = w3h[:, dk * 128:(dk + 1) * 128].T
        hb3[0:57, ai] = _f32(inp[f"{nm}3_b"])
        w4h = np.asarray(inp[f"{nm}4_w"], np.float32)                 # [A or 1, 57]
        if nm == "a":
            hw4[0:57, 0, :] = w4h.T
            hb4[:, 0] = _f32(inp["a4_b"])
        else:
            hw4[0:57, 1, 0:1] = w4h.T
            hb4c[:, 0] = _f32(inp["c4_b"])
    w["hw1"], w["hw2"], w["hw3"], w["hw4"] = map(_bf, (hw1, hw2, hw3, hw4))
    w["hb1"], w["hb2"], w["hb3"], w["hb4"], w["hb4c"] = hb1, hb2, hb3, hb4, hb4c
    return w


def prep_x(x):
    """x [B,S,F] -> per-core padded feature-major bf16 [64, BPC, 514]."""
    outs = []
    for c in range(NC):
        xp = np.zeros((F, BPC, S + 2), np.float32)
        xp[:, :, 1:S + 1] = np.asarray(
            x[c * BPC:(c + 1) * BPC], np.float32).transpose(2, 0, 1)
        outs.append(_bf(xp))
    return outs


# ---------------------------------------------------------------------------
# Bass kernel builder
# ---------------------------------------------------------------------------
def build_kernel(dbg_stage=""):
    nc = bass.Bass()

    def par(name, shape, dt=bf16):
        return nc.declare_dram_parameter(name, list(shape), dt, isOutput=False)

    xpad_d = par("xpad", [64, BPC, 514])
    c1w_d = par("c1w", [64, 3, 2, 128])
    c2w_d = par("c2w", [128, 2, 3, 4, 128])
    embw_d = par("embw", [128, 4, 4, 128])
    post_d = par("post", [128, 4, 512])
    masks_d = par("masks", [128, 5, 4, 512])
    bias0_d = par("bias0", [128, 10], f32)
    wq_d = par("wq", [L, 128, 4, 4, 128])
    wk_d = par("wk", [L, 128, 4, 4, 128])
    wv_d = par("wv", [L, 128, 4, 512])
    wo_d = par("wo", [L, 128, 4, 512])
    w1_d = par("w1", [L, 128, 4, 2048])
    w2_d = par("w2", [L, 128, 16, 512])
    lbias_d = par("lbias", [L, 128, 32], f32)
    hw1_d = par("hw1", [128, 2, 4, 128])
    hw2_d = par("hw2", [128, 2, 2, 128])
    hw3_d = par("hw3", [128, 2, 2, 64])
    hw4_d = par("hw4", [64, 2, 3])
    hb1_d = par("hb1", [128, 2], f32)
    hb2_d = par("hb2", [128, 2, 2], f32)
    hb3_d = par("hb3", [64, 2], f32)
    hb4_d = par("hb4", [3, 1], f32)
    hb4c_d = par("hb4c", [1, 1], f32)
    out_d = nc.declare_dram_parameter("out", [4, BPC], f32, isOutput=True)
    dbg_d = None
    if dbg_stage:
        dbg_d = nc.declare_dram_parameter("dbg", [128, 4, N], f32, isOutput=True)

    with tile.TileContext(nc) as tc:
        with tc.tile_pool(name="single", bufs=1) as single, \
             tc.tile_pool(name="wpool", bufs=2) as wpool, \
             tc.tile_pool(name="big1", bufs=1) as big1, \
             tc.tile_pool(name="per2", bufs=2) as per2, \
             tc.tile_pool(name="per3", bufs=3) as per3, \
             tc.tile_pool(name="ps", bufs=1, space="PSUM") as ps:

            # ---------------- persistent tiles ----------------
            x_sb = single.tile([64, BPC, 514], bf16)
            nc.sync.dma_start(out=x_sb, in_=xpad_d[:])
            c1w_t = single.tile([64, 3, 2, 128], bf16)
            nc.sync.dma_start(out=c1w_t, in_=c1w_d[:])
            c2w_t = single.tile([128, 2, 3, 4, 128], bf16)
            nc.sync.dma_start(out=c2w_t, in_=c2w_d[:])
            embw_t = single.tile([128, 4, 4, 128], bf16)
            nc.sync.dma_start(out=embw_t, in_=embw_d[:])
            pos_t = single.tile([128, 4, 512], bf16)
            nc.sync.dma_start(out=pos_t, in_=post_d[:])
            mask_t = single.tile([128, 5, 4, 512], bf16)
            nc.sync.dma_start(out=mask_t, in_=masks_d[:])
            bias0_t = single.tile([128, 10], f32)
            nc.sync.dma_start(out=bias0_t, in_=bias0_d[:])

            ones128 = single.tile([128, 1], bf16)   # stats lhsT
            nc.vector.memset(ones128, 1.0)
            ones1 = single.tile([1, 128], bf16)     # bcast lhsT
            nc.vector.memset(ones1, 1.0)
            nones = single.tile([1, 128], bf16)     # bcast lhsT * (-1/512)
            nc.vector.memset(nones, -1.0 / 512.0)
            eps1 = single.tile([1, 1], f32)
            nc.vector.memset(eps1, 1e-5)

            zt = big1.tile([128, 4, N], f32)        # residual stream
            h1_sb = big1.tile([128, 2, BPC, 514], bf16)
            nc.vector.memset(h1_sb, 0.0)
            h2_sb = big1.tile([128, 4, BPC, 512], bf16)

            # ---------------- conv feature extractor ----------------
            for s in range(BPC):
                for mc in range(2):
                    pc = ps.tile([128, 512], f32, tag="mm", bufs=3)
                    for d in range(3):
                        nc.tensor.matmul(pc, c1w_t[:, d, mc, :],
                                         x_sb[:, s, d:d + 512],
                                         start=(d == 0), stop=(d == 2))
                    nc.scalar.activation(out=h1_sb[:, mc, s, 1:513], in_=pc,
                                         func=AF.Gelu, bias=bias0_t[:, mc:mc + 1])
            for s in range(BPC):
                for mo in range(4):
                    pc = ps.tile([128, 512], f32, tag="mm", bufs=3)
                    k = 0
                    for dk in range(2):
                        for d in range(3):
                            nc.tensor.matmul(pc, c2w_t[:, dk, d, mo, :],
                                             h1_sb[:, dk, s, d:d + 512],
                                             start=(k == 0), stop=(k == 5))
                            k += 1
                    nc.scalar.activation(out=h2_sb[:, mo, s, :], in_=pc,
                                         func=AF.Gelu,
                                         bias=bias0_t[:, 2 + mo:3 + mo])
            # ---------------- input embedding ----------------
            for s in range(BPC):
                for mo in range(4):
                    pc = ps.tile([128, 512], f32, tag="mm", bufs=3)
                    for dk in range(4):
                        nc.tensor.matmul(pc, embw_t[:, dk, mo, :],
                                         h2_sb[:, dk, s, :],
                                         start=(dk == 0), stop=(dk == 3))
                    nc.vector.scalar_tensor_tensor(
                        out=zt[:, mo, s * S:(s + 1) * S], in0=pc,
                        scalar=bias0_t[:, 6 + mo:7 + mo], in1=pos_t[:, mo, :],
                        op0=OP.add, op1=OP.add)

            # ---------------- LN helper ----------------
            def layer_norm(dst_u, s):
                """LN over feature dim of zt columns of sample s -> dst_u
                (bf16 [128,4,N]); stats via ones-matmuls; rstd via ln/exp."""
                sl = slice(s * S, (s + 1) * S)
                zbf = per2.tile([128, 4, S], bf16, tag="zbf", bufs=1)
                zsq = per2.tile([128, 4, S], bf16, tag="zsq", bufs=1)
                for c in range(4):
                    nc.vector.tensor_copy(out=zbf[:, c, :], in_=zt[:, c, sl])
                    nc.scalar.activation(out=zsq[:, c, :], in_=zt[:, c, sl],
                                         func=AF.Square)
                p_sz = ps.tile([1, 512], f32, tag="st", bufs=2)
                p_sq = ps.tile([1, 512], f32, tag="st", bufs=2)
                for c in range(4):
                    nc.tensor.matmul(p_sz, ones128, zbf[:, c, :],
                                     start=(c == 0), stop=(c == 3))
                for c in range(4):
                    nc.tensor.matmul(p_sq, ones128, zsq[:, c, :],
                                     start=(c == 0), stop=(c == 3))
                t1 = per3.tile([1, 512], f32, tag="tchain", bufs=4, name="t1")
                nc.scalar.activation(out=t1, in_=p_sz, func=AF.Square)
                vr = per3.tile([1, 512], f32, tag="tchain", bufs=4, name="vr")
                nc.vector.scalar_tensor_tensor(out=vr, in0=p_sq, scalar=512.0,
                                               in1=t1, op0=OP.mult,
                                               op1=OP.subtract)
                lnv = per3.tile([1, 512], f32, tag="tchain", bufs=4, name="lnv")
                nc.scalar.activation(out=lnv, in_=vr, func=AF.Ln,
                                     scale=1.0 / (512.0 * 512.0), bias=eps1)
                s1row = per3.tile([1, 512], bf16, tag="srow", bufs=4, name="s1row")
                nc.scalar.activation(out=s1row, in_=lnv, func=AF.Exp, scale=-0.5)
                s2tmp = per3.tile([1, 512], bf16, tag="srow", bufs=4, name="s2tmp")
                nc.vector.tensor_tensor(out=s2tmp, in0=p_sz, in1=s1row,
                                        op=OP.mult)
                p_s1b = ps.tile([128, 512], f32, tag="st", bufs=2)
                nc.tensor.matmul(p_s1b, ones1, s1row, start=True, stop=True)
                p_s2b = ps.tile([128, 512], f32, tag="st", bufs=2)
                nc.tensor.matmul(p_s2b, nones, s2tmp, start=True, stop=True)
                for c in range(4):
                    nc.vector.tensor_tensor(out=dst_u[:, c, sl],
                                            in0=zt[:, c, sl],
                                            in1=p_s1b, op=OP.mult)
                    nc.vector.tensor_tensor(out=dst_u[:, c, sl],
                                            in0=dst_u[:, c, sl],
                                            in1=p_s2b, op=OP.add)

            # ---------------- transformer layers ----------------
            for l in range(L):
                wq_t = wpool.tile([128, 4, 4, 128], bf16, tag="wq", bufs=1)
                nc.sync.dma_start(out=wq_t, in_=wq_d[l])
                wk_t = wpool.tile([128, 4, 4, 128], bf16, tag="wk", bufs=1)
                nc.sync.dma_start(out=wk_t, in_=wk_d[l])
                wv_t = wpool.tile([128, 4, 512], bf16, tag="wv", bufs=1)
                nc.sync.dma_start(out=wv_t, in_=wv_d[l])
                wo_t = wpool.tile([128, 4, 512], bf16, tag="wo", bufs=1)
                nc.sync.dma_start(out=wo_t, in_=wo_d[l])
                w1_t = wpool.tile([128, 4, 2048], bf16, tag="w1", bufs=1)
                nc.sync.dma_start(out=w1_t, in_=w1_d[l])
                w2_t = wpool.tile([128, 16, 512], bf16, tag="w2", bufs=1)
                nc.sync.dma_start(out=w2_t, in_=w2_d[l])
                lb_t = wpool.tile([128, 32], f32, tag="lb", bufs=1)
                nc.sync.dma_start(out=lb_t, in_=lbias_d[l])

                u = per2.tile([128, 4, N], bf16, tag="u", bufs=1)
                u2 = per2.tile([128, 4, N], bf16, tag="u2", bufs=1)
                q_all = per2.tile([128, 4, N], bf16, tag="q", bufs=1)
                k_all = per2.tile([128, 4, N], bf16, tag="k", bufs=1)
                v_aug = per2.tile([128, 8, 8, 65], bf16, tag="vaug", bufs=1)
                nc.vector.memset(v_aug[:, :, :, 64:65], 1.0)
                o_all = per2.tile([128, 4, N], bf16, tag="oall", bufs=1)

                for s in range(BPC):
                    sl = slice(s * S, (s + 1) * S)
                    layer_norm(u, s)
                    # QKV projections for this sample
                    for dst, wt, bcol in ((q_all, wq_t, 0), (k_all, wk_t, 4)):
                        for pr in range(4):
                            pc = ps.tile([128, 512], f32, tag="mm", bufs=3)
                            for dk in range(4):
                                nc.tensor.matmul(
                                    pc, wt[:, pr, dk, :], u[:, dk, sl],
                                    start=(dk == 0), stop=(dk == 3))
                            nc.scalar.activation(
                                out=dst[:, pr, sl], in_=pc, func=AF.Identity,
                                bias=lb_t[:, bcol + pr:bcol + pr + 1])
                    for tcl in range(4):
                        tcg = s * 4 + tcl
                        pc = ps.tile([128, 512], f32, tag="mm", bufs=3)
                        for dk in range(4):
                            nc.tensor.matmul(
                                pc, u[:, dk, tcg * 128:(tcg + 1) * 128],
                                wv_t[:, dk, :], start=(dk == 0), stop=(dk == 3))
                        for h in range(H):
                            nc.vector.tensor_copy(
                                out=v_aug[:, h, tcg, 0:64],
                                in_=pc[:, h * 64:(h + 1) * 64])
                    # attention
                    for pr in range(4):
                        rts = [per3.tile([1, 512], f32, tag="rt0", name="rt0", bufs=2),
                               per3.tile([1, 512], f32, tag="rt1", name="rt1", bufs=2)]
                        rtbs = [per3.tile([1, 512], bf16, tag="rtb0", name="rtb0", bufs=2),
                                per3.tile([1, 512], bf16, tag="rtb1", name="rtb1", bufs=2)]
                        # 1/r via exp(-ln(r)) on ScalarE (custom-DVE recip is
                        # broken in this toolchain; vector.reciprocal is 3.4us)
                        pvs = []
                        for o in range(2):
                            h = 2 * pr + o
                            ob = o * 64
                            pv = ps.tile([65, 512], f32, tag="pv", bufs=3)
                            pvs.append(pv)
                            for tcl in range(4):
                                pss = ps.tile([128, 512], f32, tag="mm", bufs=3)
                                tg = s * 512 + tcl * 128
                                nc.tensor.matmul(
                                    pss,
                                    k_all[ob:ob + 64, pr, tg:tg + 128],
                                    q_all[ob:ob + 64, pr, sl],
                                    start=True, stop=True)
                                pt = per3.tile([128, 512], bf16, tag="pt")
                                nc.scalar.activation(out=pt, in_=pss,
                                                     func=AF.Exp, scale=0.125)
                                nc.vector.tensor_tensor(
                                    out=pt, in0=pt,
                                    in1=mask_t[:, min(h, 4), tcl, :],
                                    op=OP.mult)
                                nc.tensor.matmul(pv, v_aug[:, h, s * 4 + tcl, :],
                                                 pt, start=(tcl == 0),
                                                 stop=(tcl == 3))
                            nc.scalar.activation(out=rts[o], in_=pv[64:65, :],
                                                 func=AF.Ln)
                            nc.scalar.activation(out=rtbs[o], in_=rts[o],
                                                 func=AF.Exp, scale=-1.0)
                        p_b = ps.tile([128, 512], f32, tag="st", bufs=2)
                        nc.tensor.matmul(p_b[0:64, :], ones1[:, 0:64],
                                         rtbs[0], start=True, stop=True)
                        nc.tensor.matmul(p_b[64:128, :], ones1[:, 0:64],
                                         rtbs[1], start=True, stop=True)
                        b_sb = per3.tile([128, 512], bf16, tag="bsb")
                        nc.vector.tensor_copy(out=b_sb, in_=p_b)
                        for o in range(2):
                            nc.vector.tensor_tensor(
                                out=o_all[o * 64:(o + 1) * 64, pr, sl],
                                in0=pvs[o][0:64, :],
                                in1=b_sb[o * 64:(o + 1) * 64, :], op=OP.mult)
                    # output projection + residual
                    for mo in range(4):
                        pc = ps.tile([128, 512], f32, tag="mm", bufs=3)
                        for dk in range(4):
                            nc.tensor.matmul(
                                pc, wo_t[:, dk, mo * 128:(mo + 1) * 128],
                                o_all[:, dk, sl],
                                start=(dk == 0), stop=(dk == 3))
                        nc.vector.scalar_tensor_tensor(
                            out=zt[:, mo, sl], in0=pc,
                            scalar=lb_t[:, 8 + mo:9 + mo],
                            in1=zt[:, mo, sl],
                            op0=OP.add, op1=OP.add)
                    # FFN
                    layer_norm(u2, s)
                    pas = [ps.tile([128, 512], f32,
                                   tag=("pv" if mo < 2 else "st"),
                                   bufs=(3 if mo < 2 else 2),
                                   name=f"pa{mo}") for mo in range(4)]
                    for hc in range(16):
                        pc = ps.tile([128, 512], f32, tag="mm", bufs=3)
                        for dk in range(4):
                            nc.tensor.matmul(
                                pc, w1_t[:, dk, hc * 128:(hc + 1) * 128],
                                u2[:, dk, sl],
                                start=(dk == 0), stop=(dk == 3))
                        hgel = per3.tile([128, 512], bf16, tag="hgel")
                        nc.scalar.activation(out=hgel, in_=pc,
                                             func=AF.Gelu,
                                             bias=lb_t[:, 12 + hc:13 + hc])
                        for mo in range(4):
                            nc.tensor.matmul(
                                pas[mo], w2_t[:, hc, mo * 128:(mo + 1) * 128],
                                hgel, start=(hc == 0), stop=(hc == 15))
                    for mo in range(4):
                        nc.vector.scalar_tensor_tensor(
                            out=zt[:, mo, sl], in0=pas[mo],
                            scalar=lb_t[:, 28 + mo:29 + mo],
                            in1=zt[:, mo, sl],
                            op0=OP.add, op1=OP.add)
                if dbg_stage == f"l{l}":
                    dg = per2.tile([128, 4, N], f32, tag="dbg")
                    for c in range(4):
                        nc.vector.tensor_copy(out=dg[:, c, :], in_=zt[:, c, :])
                    nc.sync.dma_start(out=dbg_d[:], in_=dg)
            if dbg_stage == "emb":
                dg = per2.tile([128, 4, N], f32, tag="dbg")
                for c in range(4):
                    nc.vector.tensor_copy(out=dg[:, c, :], in_=zt[:, c, :])
                nc.sync.dma_start(out=dbg_d[:], in_=dg)

            # ---------------- final LN on last-token columns ----------------
            hw1_t = single.tile([128, 2, 4, 128], bf16)
            nc.sync.dma_start(out=hw1_t, in_=hw1_d[:])
            hw2_t = single.tile([128, 2, 2, 128], bf16)
            nc.sync.dma_start(out=hw2_t, in_=hw2_d[:])
            hw3_t = single.tile([128, 2, 2, 64], bf16)
            nc.sync.dma_start(out=hw3_t, in_=hw3_d[:])
            hw4_t = single.tile([64, 2, 3], bf16)
            nc.sync.dma_start(out=hw4_t, in_=hw4_d[:])
            hb1_t = single.tile([128, 2], f32)
            nc.sync.dma_start(out=hb1_t, in_=hb1_d[:])
            hb2_t = single.tile([128, 2, 2], f32)
            nc.sync.dma_start(out=hb2_t, in_=hb2_d[:])
            hb3_t = single.tile([64, 2], f32)
            nc.sync.dma_start(out=hb3_t, in_=hb3_d[:])
            hb4_t = single.tile([3, 1], f32)
            nc.sync.dma_start(out=hb4_t, in_=hb4_d[:])
            hb4c_t = single.tile([1, 1], f32)
            nc.sync.dma_start(out=hb4c_t, in_=hb4c_d[:])

            zf_b = single.tile([128, 4, BPC], bf16)
            zf_q = single.tile([128, 4, BPC], bf16)
            for c in range(4):
                nc.vector.tensor_copy(out=zf_b[:, c, :],
                                      in_=zt[:, c, S - 1:N:S])
                nc.scalar.activation(out=zf_q[:, c, :], in_=zt[:, c, S - 1:N:S],
                                     func=AF.Square)
            pf_sz = ps.tile([1, BPC], f32, tag="st", bufs=2)
            pf_sq = ps.tile([1, BPC], f32, tag="st", bufs=2)
            for c in range(4):
                nc.tensor.matmul(pf_sz, ones128, zf_b[:, c, :],
                                 start=(c == 0), stop=(c == 3))
            for c in range(4):
                nc.tensor.matmul(pf_sq, ones128, zf_q[:, c, :],
                                 start=(c == 0), stop=(c == 3))
            sumzf = single.tile([1, BPC], f32)
            nc.vector.tensor_copy(out=sumzf, in_=pf_sz)
            t2f = single.tile([1, BPC], f32)
            nc.vector.tensor_scalar(out=t2f, in0=pf_sq, scalar1=512.0,
                                    scalar2=None, op0=OP.mult)
            t1f = single.tile([1, BPC], f32)
            nc.vector.tensor_tensor(out=t1f, in0=sumzf, in1=sumzf, op=OP.mult)
            vrf = single.tile([1, BPC], f32)
            nc.vector.tensor_tensor(out=vrf, in0=t2f, in1=t1f, op=OP.subtract)
            lnvf = single.tile([1, BPC], f32)
            nc.scalar.activation(out=lnvf, in_=vrf, func=AF.Ln,
                                 scale=1.0 / (512.0 * 512.0), bias=eps1)
            s1f = single.tile([1, BPC], bf16)
            nc.scalar.activation(out=s1f, in_=lnvf, func=AF.Exp, scale=-0.5)
            s2f = single.tile([1, BPC], bf16)
            nc.vector.tensor_tensor(out=s2f, in0=sumzf, in1=s1f, op=OP.mult)
            pf_s1 = ps.tile([128, BPC], f32, tag="st", bufs=2)
            nc.tensor.matmul(pf_s1, ones1, s1f, start=True, stop=True)
            pf_s2 = ps.tile([128, BPC], f32, tag="st", bufs=2)
            nc.tensor.matmul(pf_s2, nones, s2f, start=True, stop=True)
            feat = single.tile([128, 4, BPC], bf16)
            for c in range(4):
                tmpf = single.tile([128, BPC], f32, tag=f"tmpf{c}")
                nc.vector.tensor_tensor(out=tmpf, in0=zt[:, c, S - 1:N:S],
                                        in1=pf_s1, op=OP.mult)
                nc.vector.tensor_tensor(out=feat[:, c, :], in0=tmpf,
                                        in1=pf_s2, op=OP.add)

            # ---------------- actor/critic heads ----------------
            outs = []
            for ai in range(2):
                pc = ps.tile([128, BPC], f32, tag="st", bufs=2)
                for dk in range(4):
                    nc.tensor.matmul(pc, hw1_t[:, ai, dk, :], feat[:, dk, :],
                                     start=(dk == 0), stop=(dk == 3))
                f1 = single.tile([128, BPC], bf16, tag=f"f1_{ai}")
                nc.scalar.activation(out=f1, in_=pc, func=AF.Gelu,
                                     bias=hb1_t[:, ai:ai + 1])
                f2 = single.tile([128, 2, BPC], bf16, tag=f"f2_{ai}")
                for mo in range(2):
                    pc2 = ps.tile([128, BPC], f32, tag="st", bufs=2)
                    nc.tensor.matmul(pc2, hw2_t[:, ai, mo, :], f1,
                                     start=True, stop=True)
                    nc.scalar.activation(out=f2[:, mo, :], in_=pc2, func=AF.Gelu,
                                         bias=hb2_t[:, ai, mo:mo + 1])
                pc3 = ps.tile([64, BPC], f32, tag="st", bufs=2)
                for dk in range(2):
                    nc.tensor.matmul(pc3, hw3_t[:, ai, dk, :], f2[:, dk, :],
                                     start=(dk == 0), stop=(dk == 1))
                f3 = single.tile([64, BPC], bf16, tag=f"f3_{ai}")
                nc.scalar.activation(out=f3, in_=pc3, func=AF.Gelu,
                                     bias=hb3_t[:, ai:ai + 1])
                outs.append(f3)
            pol_ps = ps.tile([3, BPC], f32, tag="st", bufs=2)
            nc.tensor.matmul(pol_ps, hw4_t[:, 0, :], outs[0],
                             start=True, stop=True)
            pol_sb = single.tile([3, BPC], f32)
            nc.scalar.activation(out=pol_sb, in_=pol_ps, func=AF.Identity,
                                 bias=hb4_t[:, 0:1])
            val_ps = ps.tile([1, BPC], f32, tag="st", bufs=2)
            nc.tensor.matmul(val_ps, hw4_t[:, 1, 0:1], outs[1],
                             start=True, stop=True)
            val_sb = single.tile([1, BPC], f32)
            nc.scalar.activation(out=val_sb, in_=val_ps, func=AF.Identity,
                                 bias=hb4c_t[:, 0:1])
            nc.sync.dma_start(out=out_d[0:3, :], in_=pol_sb)
            nc.sync.dma_start(out=out_d[3:4, :], in_=val_sb)

    n = split_sync_waits(nc)
    return nc


# ---------------------------------------------------------------------------
# Entry point
# ---------------------------------------------------------------------------
_CACHE = {}


def kernel(**inputs):
    dbg_stage = os.environ.get("BASS_DBG_STAGE", "")
    key = ("nc", dbg_stage)
    if key not in _CACHE:
        _CACHE[key] = build_kernel(dbg_stage)
    nc = _CACHE[key]
    w = preprocess(inputs)
    xs = prep_x(np.asarray(inputs["x"], np.float32))
    in_maps = []
    for c in range(NC):
        m = {"xpad": xs[c]}
        m.update(w)
        in_maps.append(m)
    trace = os.environ.get("BASS_KERNEL_TRACE", "") == "1"
    res = run_bass_kernel_spmd(nc, in_maps, core_ids=list(range(NC)),
                               trace=trace)
    kernel.last_result = res
    policy = np.zeros((B, A), np.float32)
    value = np.zeros((B, 1), np.float32)
    for c in range(NC):
        o = np.asarray(res.results[c]["out"], np.float32)
        policy[c * BPC:(c + 1) * BPC] = o[0:3].T
        value[c * BPC:(c + 1) * BPC] = o[3:4].T
    if dbg_stage:
        kernel.dbg = [np.asarray(res.results[c]["dbg"]) for c in range(NC)]
    return policy, value
